# revision 13
# baseline (speedup 1.0000x reference)
"""YOLO-style loss kernel for Trainium2, SPMD over 8 NeuronCores.

Inputs (full): pred_tensor [32768,7,7,30] f32, target_tensor [32768,7,7,30] f32.
Output: np.ndarray shape (5,) f32 = (loss_xy, loss_wh, loss_obj, loss_noobj, loss_class).

Strategy: pure data parallel on batch dim; each core gets 4096 samples
(200704 cells). Host converts to fp16 and regroups channels so every hot
on-chip op is a dense step-1 access (DVE 2x packed mode):
  - pred boxes  [n,10] cell-major as (x0,y0,x1,y1, w0,h0,w1,h1, c0,c1)
  - tgt  boxes  [n,10] cell-major as (x0,y0,w0,h0, x1,y1,w1,h1, c0,c1)
  - classes     [20,n] channel-major per chunk (both tensors)
Per 392-cell chunk: IoU responsibility + five masked squared-diff partial
sums, fused on-chip; squares/copies run on the scalar engine, reciprocal via
the ~1cpe approx custom-DVE op, weighted reductions via stt accum columns.
Each core returns a [128, 20] f32 partial-sum tile (5 losses x 4 chunks);
host reduces and divides by N.
"""

import os
import sys

sys.path.insert(0, "/opt/trn_rl_repo")

import numpy as np

import concourse.bass as bass
import concourse.bacc as bacc
import concourse.tile as tile
from concourse import mybir
from concourse import bass_utils

F32 = mybir.dt.float32
F16 = mybir.dt.float16
ALU = mybir.AluOpType
ACT = mybir.ActivationFunctionType

S = 7
B = 2
C = 20
D = 30
N_FULL = 32768
N_CORES = 8
N_SHARD = N_FULL // N_CORES            # 4096 samples per core
R = N_SHARD * S * S                    # 200704 cells per core
P = 128                                # partitions
RP = R // P                            # 1568 cells per partition
NCK = 392                              # cells per partition per chunk
N_CH = RP // NCK                       # 4 chunks

# channel permutations applied on host (fp16 cast + gather)
PERM_P = [0, 1, 5, 6, 2, 3, 7, 8, 4, 9]   # pred:  x0,y0,x1,y1,w0,h0,w1,h1,c0,c1
PERM_T = [0, 1, 2, 3, 5, 6, 7, 8, 4, 9]   # tgt:   x0,y0,w0,h0,x1,y1,w1,h1,c0,c1


def _ins(ap, pos, step, count):
    """Insert a [step, count] dim at free-dim position `pos` (absolute index
    into the ap list, where index 0 is the partition dim)."""
    new = [list(x) for x in ap.ap]
    new.insert(pos, [step, count])
    return bass.AP(tensor=ap.tensor, offset=ap.offset, ap=new)


def _mk(ap, dims):
    """Rebuild the free dims of `ap` (keeping partition dim + offset) as
    `dims` = list of (step, count)."""
    new = [list(ap.ap[0])] + [[s, c] for s, c in dims]
    return bass.AP(tensor=ap.tensor, offset=ap.offset, ap=new)


def build_program():
    nc = bacc.Bacc("TRN2", target_bir_lowering=False, debug=False)
    n = NCK

    pbox = nc.dram_tensor("pbox", [P, N_CH * n * 10], F16, kind="ExternalInput")
    tbox = nc.dram_tensor("tbox", [P, N_CH * n * 10], F16, kind="ExternalInput")
    pcls = nc.dram_tensor("pcls", [P, N_CH * C * n], F16, kind="ExternalInput")
    tcls = nc.dram_tensor("tcls", [P, N_CH * C * n], F16, kind="ExternalInput")
    out = nc.dram_tensor("out", [P, 5 * N_CH], F32, kind="ExternalOutput")

    pbox_v = pbox.ap().rearrange("p (k n c) -> p k n c", k=N_CH, n=n, c=10)
    tbox_v = tbox.ap().rearrange("p (k n c) -> p k n c", k=N_CH, n=n, c=10)
    pcls_v = pcls.ap().rearrange("p (k c i) -> p k c i", k=N_CH, c=C, i=n)
    tcls_v = tcls.ap().rearrange("p (k c i) -> p k c i", k=N_CH, c=C, i=n)

    with tile.TileContext(nc) as tc:
        with (
            tc.tile_pool(name="raw", bufs=2) as raw,
            tc.tile_pool(name="tmp", bufs=1) as tmp,
            tc.tile_pool(name="persist", bufs=1) as persist,
        ):
            acc = persist.tile([P, 5 * N_CH], F32)

            for k in range(N_CH):
                Pb = raw.tile([P, n, 10], F16, tag="Pb")
                Tb = raw.tile([P, n, 10], F16, tag="Tb")
                Pc = raw.tile([P, C, n], F16, tag="Pc")
                Tc = raw.tile([P, C, n], F16, tag="Tc")
                nc.sync.dma_start(out=Pb, in_=pbox_v[:, k])
                nc.sync.dma_start(out=Tb, in_=tbox_v[:, k])
                nc.sync.dma_start(out=Pc, in_=pcls_v[:, k])
                nc.sync.dma_start(out=Tc, in_=tcls_v[:, k])

                # ---- views ----
                pxy4 = Pb[:, :, 0:4]           # (x0,y0,x1,y1) step1
                pwh4 = Pb[:, :, 4:8]           # (w0,h0,w1,h1) step1
                pc2 = Pb[:, :, 8:10]
                txy0 = Tb[:, :, 0:2]
                twh0 = Tb[:, :, 2:4]
                tc2 = Tb[:, :, 8:10]
                obj_src = Tb[:, :, 8]          # [P,n] step10
                # target (x0,y0,x1,y1): [n][box step4][coord step1]
                txy4v = _ins(Tb[:, :, 0:2], 2, 4, 2)
                # target (w0,h0,w1,h1): same with offset 2
                twh4v = _ins(Tb[:, :, 2:4], 2, 4, 2)

                def sqacc(dm, col):
                    # in-place square: ACT streams read-then-write per element,
                    # so out == in is safe and avoids junk tiles whose reuse
                    # would couple engines across chunks
                    nc.scalar.activation(
                        dm, dm, ACT.Square,
                        accum_out=acc[:, 5 * k + col:5 * k + col + 1],
                    )

                # class loss first: its ACT square-accum (6.8us) then overlaps
                # the DVE IoU stage, and next chunk's dcl/dmcl writes no longer
                # wait on a late ACT read
                # class (channel-major [P,20,n]). The obj premask would be a 1x
                # broadcast multiply (8.3us); instead AND the fp16 diffs against
                # a 0xFFFF/0x0000 mask through int32-reinterpreted views — the
                # pair-packing halves the element count (4.25us).
                ffi = tmp.tile([P, n], mybir.dt.int16, tag="ffi")
                nc.scalar.activation(ffi, obj_src, ACT.Copy, scale=-1.0)  # -1 -> 0xFFFF
                ff32 = ffi.bitcast(mybir.dt.int32)                        # [P, n/2]
                ff32b = _mk(ff32[:, 0], [(0, C), (1, n // 2)])

                dcl = tmp.tile([P, C, n], F16, tag="dcl")
                dmcl = tmp.tile([P, C, n], F16, tag="dmcl")
                nc.vector.tensor_tensor(dcl, Tc, Pc, op=ALU.subtract)
                nc.vector.tensor_tensor(
                    dmcl.bitcast(mybir.dt.int32), dcl.bitcast(mybir.dt.int32), ff32b,
                    op=ALU.bitwise_and,
                )
                sqacc(dmcl, 4)

                # ---- IoU stage (coords scaled x7: corners 3.5*wh -+ xy) ----
                # ts+tt instead of scalar_tensor_tensor: stt measures 2 cyc/elem
                # on this HW, ts+tt is ~0.5+0.5
                t1 = tmp.tile([P, n, 4], F16, tag="t1")
                nc.vector.tensor_scalar(t1, pwh4, 3.5, None, op0=ALU.mult)
                nl4 = tmp.tile([P, n, 4], F16, tag="nl4")    # -(7l) both boxes
                r4 = tmp.tile([P, n, 4], F16, tag="r4")      # 7r both boxes
                nc.vector.tensor_tensor(nl4, t1, pxy4, op=ALU.subtract)
                nc.vector.tensor_tensor(r4, t1, pxy4, op=ALU.add)

                t2 = tmp.tile([P, n, 2], F16, tag="t2")
                nc.vector.tensor_scalar(t2, twh0, 3.5, None, op0=ALU.mult)
                nlt2 = tmp.tile([P, n, 2], F16, tag="nlt2")
                rt2 = tmp.tile([P, n, 2], F16, tag="rt2")
                nc.vector.tensor_tensor(nlt2, t2, txy0, op=ALU.subtract)
                nc.vector.tensor_tensor(rt2, t2, txy0, op=ALU.add)
                # broadcast target corners over box dim: [n][box step0][coord step1]
                nlt2b = _ins(nlt2[:, :, :], 2, 0, 2)
                rt2b = _ins(rt2[:, :, :], 2, 0, 2)

                mln4 = tmp.tile([P, n, 4], F16, tag="mln4")
                mr4 = tmp.tile([P, n, 4], F16, tag="mr4")
                nc.vector.tensor_tensor(mln4, nl4, nlt2b, op=ALU.min)
                nc.vector.tensor_tensor(mr4, r4, rt2b, op=ALU.min)
                s4 = nl4  # dead, reuse
                nc.vector.tensor_tensor(s4, mln4, mr4, op=ALU.add)   # 7*(minr-maxl)
                cw4 = r4  # dead, reuse
                nc.vector.tensor_scalar(cw4, s4, 1.0 / 7.0, 0.0, op0=ALU.mult, op1=ALU.max)

                # per-box scalars, box-major [P,2,n]
                inter2 = tmp.tile([P, 2, n], F16, tag="inter2")
                areap2 = tmp.tile([P, 2, n], F16, tag="areap2")
                areat = tmp.tile([P, n], F16, tag="areat")
                # cw x/y lanes: [box step2][cell step4] from cw4 (x0,y0,x1,y1)
                cwx = _mk(cw4[:, :, 0], [(2, 2), (4, n)])
                cwy = _mk(cw4[:, :, 1], [(2, 2), (4, n)])
                nc.vector.tensor_tensor(inter2, cwx, cwy, op=ALU.mult)
                pw2 = _mk(Pb[:, :, 4], [(2, 2), (10, n)])
                ph2 = _mk(Pb[:, :, 5], [(2, 2), (10, n)])
                nc.vector.tensor_tensor(areap2, pw2, ph2, op=ALU.mult)
                nc.vector.tensor_tensor(areat, Tb[:, :, 2], Tb[:, :, 3], op=ALU.mult)

                u2h = tmp.tile([P, 2, n], F16, tag="u2h")
                u2 = tmp.tile([P, 2, n], F32, tag="u2")
                nc.vector.tensor_tensor(u2h, areap2, inter2, op=ALU.subtract)
                areatb = _ins(areat[:, :], 1, 0, 2)          # [box step0][cell step1]
                nc.vector.tensor_tensor(u2, u2h, areatb, op=ALU.add)

                rcp2 = tmp.tile([P, 2, n], F32, tag="rcp2")
                nc.vector.reciprocal_approx_fast(rcp2, u2)
                # fp32 operand would drop the iou multiply to ~1571ns; a scalar
                # engine downcast keeps the DVE op at 2x (~361ns)
                rcp16 = tmp.tile([P, 2, n], F16, tag="rcp16")
                nc.scalar.activation(rcp16, rcp2, ACT.Copy)
                iou2 = tmp.tile([P, 2, n], F16, tag="iou2")
                nc.vector.tensor_tensor(iou2, inter2, rcp16, op=ALU.mult)

                is1 = tmp.tile([P, n], F16, tag="is1")
                riou = tmp.tile([P, n], F16, tag="riou")
                nc.vector.tensor_tensor(is1, iou2[:, 1, :], iou2[:, 0, :], op=ALU.is_gt)
                nc.vector.tensor_tensor(riou, iou2[:, 1, :], iou2[:, 0, :], op=ALU.max)

                # obj compact copy (scalar engine) — keeps resp ops at 2x
                obj_c = tmp.tile([P, n], F16, tag="obj_c")
                nc.scalar.activation(obj_c, obj_src, ACT.Copy)

                resp = tmp.tile([P, 2, n], F16, tag="resp")
                nc.vector.tensor_tensor(resp[:, 1, :], obj_c, is1, op=ALU.mult)
                nc.vector.tensor_tensor(resp[:, 0, :], obj_c, resp[:, 1, :], op=ALU.subtract)

                # (r0,r0,r1,r1) per cell, materialized compact so the xy/wh
                # premask multiplies run at 2x
                resp4m = tmp.tile([P, n, 4], F16, tag="resp4m")
                resp4v = _mk(resp[:, 0, 0], [(1, n), (n, 2), (0, 2)])
                nc.scalar.activation(resp4m, resp4v, ACT.Copy)

                # ---- losses: premask (DVE/gpsimd) + Square-with-accum (ACT).
                # All masks are exactly 0/1, so sum(mask*d^2) == sum((mask*d)^2).

                # xy
                dxy4 = tmp.tile([P, n, 4], F16, tag="dxy4")
                dm4 = tmp.tile([P, n, 4], F16, tag="dm4")
                nc.vector.tensor_tensor(dxy4, txy4v, pxy4, op=ALU.subtract)
                nc.vector.tensor_tensor(dm4, dxy4, resp4m, op=ALU.mult)
                sqacc(dm4, 0)

                # wh (sqrt space)
                sp4 = tmp.tile([P, n, 4], F16, tag="sp4")
                st4 = tmp.tile([P, n, 4], F16, tag="st4")
                nc.scalar.activation(sp4, pwh4, ACT.Sqrt)
                nc.scalar.activation(st4, twh4v, ACT.Sqrt)
                # dxy4's last reader is the dm4 multiply (DVE) — same-engine reuse
                dwh4 = dxy4
                dmw4 = mr4  # dead, reuse
                nc.vector.tensor_tensor(dwh4, st4, sp4, op=ALU.subtract)
                nc.vector.tensor_tensor(dmw4, dwh4, resp4m, op=ALU.mult)
                sqacc(dmw4, 1)

                # obj conf vs responsible-iou, box-major [P,2,n]: the diff is 1x
                # (broadcast riou), the premask is all-compact 2x against resp
                dc2 = tmp.tile([P, 2, n], F16, tag="dc2")
                dmc2 = tmp.tile([P, 2, n], F16, tag="dmc2")
                rioub = _ins(riou[:, :], 1, 0, 2)                 # [boxdup][cell]
                pc2bm = _mk(Pb[:, 0, 8], [(1, 2), (10, n)])       # [box][cell]
                nc.vector.tensor_tensor(dc2, rioub, pc2bm, op=ALU.subtract)
                nc.vector.tensor_tensor(dmc2, dc2, resp, op=ALU.mult)
                sqacc(dmc2, 2)

                # noobj conf: noobj*(tc-pc)^2 == ((tc*pc)-pc)^2 since tc in {0,1}
                # -> all operands compact, both multiplies 2x, no mask tile
                m2 = tmp.tile([P, n, 2], F16, tag="m2")
                dmn2 = tmp.tile([P, n, 2], F16, tag="dmn2")
                nc.vector.tensor_tensor(m2, tc2, pc2, op=ALU.mult)
                nc.vector.tensor_tensor(dmn2, m2, pc2, op=ALU.subtract)
                sqacc(dmn2, 3)


            nc.sync.dma_start(out=out.ap(), in_=acc)

    nc.compile()
    return nc


_nc_cache = None
LAST_EXEC_NS = None
LAST_RESULT = None


def _get_nc():
    global _nc_cache
    if _nc_cache is None:
        _nc_cache = build_program()
    return _nc_cache


def _prep(full, perm):
    """[N*S*S, 30] f32 -> per-core (box [P, N_CH*n*10], cls [P, N_CH*20*n]) f16."""
    A = np.asarray(full, dtype=np.float32).reshape(N_CORES, P, N_CH, NCK, D)
    A16 = A.astype(np.float16)
    box = np.ascontiguousarray(A16[..., perm]).reshape(N_CORES, P, -1)
    cls_ = np.ascontiguousarray(A16[..., 10:30].transpose(0, 1, 2, 4, 3)).reshape(
        N_CORES, P, -1
    )
    return box, cls_


def kernel(pred_tensor, target_tensor):
    global LAST_EXEC_NS, LAST_RESULT
    pred = np.asarray(pred_tensor).reshape(N_FULL * S * S, D)
    tgt = np.asarray(target_tensor).reshape(N_FULL * S * S, D)

    pb, pc = _prep(pred, PERM_P)
    tb, tc = _prep(tgt, PERM_T)

    in_maps = []
    for i in range(N_CORES):
        in_maps.append({"pbox": pb[i], "tbox": tb[i], "pcls": pc[i], "tcls": tc[i]})

    nc = _get_nc()
    trace = bool(os.environ.get("KERNEL_TRACE"))
    tmpdir = os.environ.get("KERNEL_TRACE_DIR") or None
    res = bass_utils.run_bass_kernel_spmd(
        nc, in_maps, core_ids=list(range(N_CORES)), trace=trace, tmpdir=tmpdir
    )
    LAST_RESULT = res
    if res.exec_time_ns is not None:
        LAST_EXEC_NS = res.exec_time_ns
    total = np.zeros(5, dtype=np.float64)
    for m in res.results:
        total += m["out"].astype(np.float64).sum(axis=0).reshape(N_CH, 5).sum(axis=0)
    losses = (total / float(N_FULL)).astype(np.float32)
    return losses


# revision 15
# speedup vs baseline: 1.0798x; 1.0798x over previous
"""YOLO-style loss kernel for Trainium2, SPMD over 8 NeuronCores.

Inputs (full): pred_tensor [32768,7,7,30] f32, target_tensor [32768,7,7,30] f32.
Output: np.ndarray shape (5,) f32 = (loss_xy, loss_wh, loss_obj, loss_noobj, loss_class).

Strategy: pure data parallel on batch dim; each core gets 4096 samples
(200704 cells). Host converts to fp16 and splits channels into fully
contiguous groups so the hot DVE ops coalesce into the 2x packed mode
(strided views of an interleaved [n,10] tile measure 1x or worse):
  - xy4  [n,4] cell-major (x0,y0,x1,y1)        both tensors
  - wh4  [n,4] cell-major (w0,h0,w1,h1)        both tensors
  - cf2  [n,2] cell-major (c0,c1)              both tensors
  - cls  [20,n] channel-major per chunk        both tensors
Per 392-cell chunk: IoU responsibility + five masked squared-diff partial
sums, fused on-chip. Weighted reductions run as premask-multiply (masks are
exactly 0/1) + in-place Square with accum_out on the scalar engine; the
class premask ANDs int32-reinterpreted fp16 pairs against a 0xFFFF mask;
reciprocal via the ~1cpe approx custom-DVE op. Each core returns a [128,20]
f32 partial-sum tile (5 losses x 4 chunks); host reduces and divides by N.
"""

import os
import sys

sys.path.insert(0, "/opt/trn_rl_repo")

import numpy as np

import concourse.bass as bass
import concourse.bacc as bacc
import concourse.tile as tile
from concourse import mybir
from concourse import bass_utils

F32 = mybir.dt.float32
F16 = mybir.dt.float16
I16 = mybir.dt.int16
I32 = mybir.dt.int32
ALU = mybir.AluOpType
ACT = mybir.ActivationFunctionType

S = 7
B = 2
C = 20
D = 30
N_FULL = 32768
N_CORES = 8
N_SHARD = N_FULL // N_CORES            # 4096 samples per core
R = N_SHARD * S * S                    # 200704 cells per core
P = 128                                # partitions
RP = R // P                            # 1568 cells per partition
NCK = 392                              # cells per partition per chunk
N_CH = RP // NCK                       # 4 chunks

PERM_XY = [0, 1, 5, 6]   # x0,y0,x1,y1
PERM_WH = [2, 3, 7, 8]   # w0,h0,w1,h1
PERM_CF = [4, 9]         # c0,c1


def _mk(ap, dims):
    """Rebuild the free dims of `ap` (keeping partition dim + offset) as
    `dims` = list of (step, count)."""
    new = [list(ap.ap[0])] + [[s, c] for s, c in dims]
    return bass.AP(tensor=ap.tensor, offset=ap.offset, ap=new)


def _ins(ap, pos, step, count):
    new = [list(x) for x in ap.ap]
    new.insert(pos, [step, count])
    return bass.AP(tensor=ap.tensor, offset=ap.offset, ap=new)


def build_program():
    nc = bacc.Bacc("TRN2", target_bir_lowering=False, debug=False)
    n = NCK

    def din(name, per_chunk):
        return nc.dram_tensor(name, [P, N_CH * per_chunk], F16, kind="ExternalInput")

    pxy, pwh, pcf, pcl = din("pxy", n * 4), din("pwh", n * 4), din("pcf", n * 2), din("pcl", C * n)
    txy, twh, tcf, tcl = din("txy", n * 4), din("twh", n * 4), din("tcf", n * 2), din("tcl", C * n)
    out = nc.dram_tensor("out", [P, 5 * N_CH], F32, kind="ExternalOutput")

    def view(t, shape_str, **kw):
        return t.ap().rearrange(shape_str, k=N_CH, **kw)

    pxy_v = view(pxy, "p (k n c) -> p k n c", n=n, c=4)
    pwh_v = view(pwh, "p (k n c) -> p k n c", n=n, c=4)
    pcf_v = view(pcf, "p (k n c) -> p k n c", n=n, c=2)
    pcl_v = view(pcl, "p (k c i) -> p k c i", c=C, i=n)
    txy_v = view(txy, "p (k n c) -> p k n c", n=n, c=4)
    twh_v = view(twh, "p (k n c) -> p k n c", n=n, c=4)
    tcf_v = view(tcf, "p (k n c) -> p k n c", n=n, c=2)
    tcl_v = view(tcl, "p (k c i) -> p k c i", c=C, i=n)

    with tile.TileContext(nc) as tc:
        with (
            tc.tile_pool(name="raw", bufs=2) as raw,
            tc.tile_pool(name="tmp", bufs=1) as tmp,
            tc.tile_pool(name="persist", bufs=1) as persist,
        ):
            acc = persist.tile([P, 5 * N_CH], F32)

            for k in range(N_CH):
                Pxy = raw.tile([P, n, 4], F16, tag="Pxy")
                Pwh = raw.tile([P, n, 4], F16, tag="Pwh")
                Pcf = raw.tile([P, n, 2], F16, tag="Pcf")
                Pcl = raw.tile([P, C, n], F16, tag="Pcl")
                Txy = raw.tile([P, n, 4], F16, tag="Txy")
                Twh = raw.tile([P, n, 4], F16, tag="Twh")
                Tcf = raw.tile([P, n, 2], F16, tag="Tcf")
                Tcl = raw.tile([P, C, n], F16, tag="Tcl")
                nc.sync.dma_start(out=Pxy, in_=pxy_v[:, k])
                nc.sync.dma_start(out=Pwh, in_=pwh_v[:, k])
                nc.sync.dma_start(out=Pcf, in_=pcf_v[:, k])
                nc.sync.dma_start(out=Pcl, in_=pcl_v[:, k])
                nc.sync.dma_start(out=Txy, in_=txy_v[:, k])
                nc.sync.dma_start(out=Twh, in_=twh_v[:, k])
                nc.sync.dma_start(out=Tcf, in_=tcf_v[:, k])
                nc.sync.dma_start(out=Tcl, in_=tcl_v[:, k])

                obj_src = Tcf[:, :, 0]          # [P,n] step2

                def sqacc(dm, col):
                    # in-place square: ACT streams read-then-write per element,
                    # so out == in is safe and avoids junk tiles whose reuse
                    # would couple engines across chunks
                    nc.scalar.activation(
                        dm, dm, ACT.Square,
                        accum_out=acc[:, 5 * k + col:5 * k + col + 1],
                    )

                # ---- IoU stage (coords scaled x7: corners 3.5*wh -+ xy) ----
                t1 = tmp.tile([P, n, 4], F16, tag="t1")
                nc.vector.tensor_scalar(t1, Pwh, 3.5, None, op0=ALU.mult)
                nl4 = tmp.tile([P, n, 4], F16, tag="nl4")    # -(7l) both boxes
                r4 = tmp.tile([P, n, 4], F16, tag="r4")      # 7r both boxes
                nc.vector.tensor_tensor(nl4, t1, Pxy, op=ALU.subtract)
                nc.vector.tensor_tensor(r4, t1, Pxy, op=ALU.add)

                # target corners for both boxes (all-contiguous 2x); only
                # box0's lanes are consumed, broadcast over the pred box dim
                t2 = tmp.tile([P, n, 4], F16, tag="t2")
                nc.vector.tensor_scalar(t2, Twh, 3.5, None, op0=ALU.mult)
                nlt4 = tmp.tile([P, n, 4], F16, tag="nlt4")
                rt4 = tmp.tile([P, n, 4], F16, tag="rt4")
                nc.vector.tensor_tensor(nlt4, t2, Txy, op=ALU.subtract)
                nc.vector.tensor_tensor(rt4, t2, Txy, op=ALU.add)
                nlt2b = _mk(nlt4[:, 0, 0], [(4, n), (0, 2), (1, 2)])
                rt2b = _mk(rt4[:, 0, 0], [(4, n), (0, 2), (1, 2)])

                mln4 = tmp.tile([P, n, 4], F16, tag="mln4")
                mr4 = tmp.tile([P, n, 4], F16, tag="mr4")
                nc.vector.tensor_tensor(mln4, nl4, nlt2b, op=ALU.min)
                nc.vector.tensor_tensor(mr4, r4, rt2b, op=ALU.min)
                s4 = nl4  # dead, reuse
                nc.vector.tensor_tensor(s4, mln4, mr4, op=ALU.add)   # 7*(minr-maxl)
                cw4 = r4  # dead, reuse
                nc.vector.tensor_scalar(cw4, s4, 1.0 / 7.0, 0.0, op0=ALU.mult, op1=ALU.max)

                # per-box scalars, box-major [P,2,n]
                inter2 = tmp.tile([P, 2, n], F16, tag="inter2")
                areap2 = tmp.tile([P, 2, n], F16, tag="areap2")
                areat = tmp.tile([P, n], F16, tag="areat")
                cwx = _mk(cw4[:, 0, 0], [(2, 2), (4, n)])
                cwy = _mk(cw4[:, 0, 1], [(2, 2), (4, n)])
                nc.vector.tensor_tensor(inter2, cwx, cwy, op=ALU.mult)
                pw2 = _mk(Pwh[:, 0, 0], [(2, 2), (4, n)])
                ph2 = _mk(Pwh[:, 0, 1], [(2, 2), (4, n)])
                nc.vector.tensor_tensor(areap2, pw2, ph2, op=ALU.mult)
                nc.vector.tensor_tensor(areat, Twh[:, :, 0], Twh[:, :, 1], op=ALU.mult)

                u2h = tmp.tile([P, 2, n], F16, tag="u2h")
                u2 = tmp.tile([P, 2, n], F32, tag="u2")
                nc.vector.tensor_tensor(u2h, areap2, inter2, op=ALU.subtract)
                areatb = _ins(areat[:, :], 1, 0, 2)          # [box step0][cell step1]
                nc.vector.tensor_tensor(u2, u2h, areatb, op=ALU.add)

                rcp2 = tmp.tile([P, 2, n], F32, tag="rcp2")
                nc.vector.reciprocal_approx_fast(rcp2, u2)
                rcp16 = tmp.tile([P, 2, n], F16, tag="rcp16")
                nc.scalar.activation(rcp16, rcp2, ACT.Copy)
                iou2 = tmp.tile([P, 2, n], F16, tag="iou2")
                nc.vector.tensor_tensor(iou2, inter2, rcp16, op=ALU.mult)

                is1 = tmp.tile([P, n], F16, tag="is1")
                riou = tmp.tile([P, n], F16, tag="riou")
                nc.vector.tensor_tensor(is1, iou2[:, 1, :], iou2[:, 0, :], op=ALU.is_gt)
                nc.vector.tensor_tensor(riou, iou2[:, 1, :], iou2[:, 0, :], op=ALU.max)

                obj_c = tmp.tile([P, n], F16, tag="obj_c")
                nc.scalar.activation(obj_c, obj_src, ACT.Copy)

                resp = tmp.tile([P, 2, n], F16, tag="resp")
                nc.vector.tensor_tensor(resp[:, 1, :], obj_c, is1, op=ALU.mult)
                nc.vector.tensor_tensor(resp[:, 0, :], obj_c, resp[:, 1, :], op=ALU.subtract)

                # (r0,r0,r1,r1) per cell, compact so xy/wh premasks run at 2x
                resp4m = tmp.tile([P, n, 4], F16, tag="resp4m")
                resp4v = _mk(resp[:, 0, 0], [(1, n), (n, 2), (0, 2)])
                nc.scalar.activation(resp4m, resp4v, ACT.Copy)

                # ---- losses: premask (DVE) + in-place Square-with-accum (ACT)
                # xy
                dxy4 = tmp.tile([P, n, 4], F16, tag="dxy4")
                dm4 = tmp.tile([P, n, 4], F16, tag="dm4")
                nc.vector.tensor_tensor(dxy4, Txy, Pxy, op=ALU.subtract)
                nc.vector.tensor_tensor(dm4, dxy4, resp4m, op=ALU.mult)
                sqacc(dm4, 0)

                # wh (sqrt space)
                sp4 = tmp.tile([P, n, 4], F16, tag="sp4")
                st4 = tmp.tile([P, n, 4], F16, tag="st4")
                nc.scalar.activation(sp4, Pwh, ACT.Sqrt)
                nc.scalar.activation(st4, Twh, ACT.Sqrt)
                # dxy4's last reader is the dm4 multiply (DVE) — same-engine reuse
                dwh4 = dxy4
                dmw4 = mr4  # dead, reuse
                nc.vector.tensor_tensor(dwh4, st4, sp4, op=ALU.subtract)
                nc.vector.tensor_tensor(dmw4, dwh4, resp4m, op=ALU.mult)
                sqacc(dmw4, 1)

                # obj conf vs responsible-iou, box-major [P,2,n]: diff is 1x
                # (broadcast riou), premask is all-compact 2x against resp
                dc2 = tmp.tile([P, 2, n], F16, tag="dc2")
                dmc2 = tmp.tile([P, 2, n], F16, tag="dmc2")
                rioub = _ins(riou[:, :], 1, 0, 2)                 # [boxdup][cell]
                pc2bm = _mk(Pcf[:, 0, 0], [(1, 2), (2, n)])       # [box][cell]
                nc.vector.tensor_tensor(dc2, rioub, pc2bm, op=ALU.subtract)
                nc.vector.tensor_tensor(dmc2, dc2, resp, op=ALU.mult)
                sqacc(dmc2, 2)

                # noobj conf: noobj*(tc-pc)^2 == ((tc*pc)-pc)^2 since tc in {0,1}
                m2 = tmp.tile([P, n, 2], F16, tag="m2")
                dmn2 = tmp.tile([P, n, 2], F16, tag="dmn2")
                nc.vector.tensor_tensor(m2, Tcf, Pcf, op=ALU.mult)
                nc.vector.tensor_tensor(dmn2, m2, Pcf, op=ALU.subtract)
                sqacc(dmn2, 3)

                # class (channel-major [P,20,n]): AND the fp16 diffs against a
                # 0xFFFF/0x0000 obj mask through int32-reinterpreted views —
                # pair-packing halves the element count vs a 1x broadcast mult
                ffi = tmp.tile([P, n], I16, tag="ffi")
                nc.scalar.activation(ffi, obj_src, ACT.Copy, scale=-1.0)  # -1 -> 0xFFFF
                ff32 = ffi.bitcast(I32)                                   # [P, n/2]
                ff32b = _mk(ff32[:, 0], [(0, C), (1, n // 2)])

                dcl = tmp.tile([P, C, n], F16, tag="dcl")
                dmcl = tmp.tile([P, C, n], F16, tag="dmcl")
                nc.vector.tensor_tensor(dcl, Tcl, Pcl, op=ALU.subtract)
                nc.vector.tensor_tensor(
                    dmcl.bitcast(I32), dcl.bitcast(I32), ff32b, op=ALU.bitwise_and
                )
                sqacc(dmcl, 4)

            nc.sync.dma_start(out=out.ap(), in_=acc)

    nc.compile()
    return nc


_nc_cache = None
LAST_EXEC_NS = None
LAST_RESULT = None


def _get_nc():
    global _nc_cache
    if _nc_cache is None:
        _nc_cache = build_program()
    return _nc_cache


def _prep(full):
    """[N*S*S, 30] f32 -> per-core contiguous fp16 groups."""
    A = np.asarray(full, dtype=np.float32).reshape(N_CORES, P, N_CH, NCK, D)
    A16 = A.astype(np.float16)
    xy = np.ascontiguousarray(A16[..., PERM_XY]).reshape(N_CORES, P, -1)
    wh = np.ascontiguousarray(A16[..., PERM_WH]).reshape(N_CORES, P, -1)
    cf = np.ascontiguousarray(A16[..., PERM_CF]).reshape(N_CORES, P, -1)
    cl = np.ascontiguousarray(A16[..., 10:30].transpose(0, 1, 2, 4, 3)).reshape(
        N_CORES, P, -1
    )
    return xy, wh, cf, cl


def kernel(pred_tensor, target_tensor):
    global LAST_EXEC_NS, LAST_RESULT
    pred = np.asarray(pred_tensor).reshape(N_FULL * S * S, D)
    tgt = np.asarray(target_tensor).reshape(N_FULL * S * S, D)

    pxy, pwh, pcf, pcl = _prep(pred)
    txy, twh, tcf, tcl = _prep(tgt)

    in_maps = []
    for i in range(N_CORES):
        in_maps.append({
            "pxy": pxy[i], "pwh": pwh[i], "pcf": pcf[i], "pcl": pcl[i],
            "txy": txy[i], "twh": twh[i], "tcf": tcf[i], "tcl": tcl[i],
        })

    nc = _get_nc()
    trace = bool(os.environ.get("KERNEL_TRACE"))
    tmpdir = os.environ.get("KERNEL_TRACE_DIR") or None
    res = bass_utils.run_bass_kernel_spmd(
        nc, in_maps, core_ids=list(range(N_CORES)), trace=trace, tmpdir=tmpdir
    )
    LAST_RESULT = res
    if res.exec_time_ns is not None:
        LAST_EXEC_NS = res.exec_time_ns
    total = np.zeros(5, dtype=np.float64)
    for m in res.results:
        total += m["out"].astype(np.float64).sum(axis=0).reshape(N_CH, 5).sum(axis=0)
    losses = (total / float(N_FULL)).astype(np.float32)
    return losses


# revision 18
# speedup vs baseline: 1.1417x; 1.0573x over previous
"""YOLO-style loss kernel for Trainium2, SPMD over 8 NeuronCores.

Inputs (full): pred_tensor [32768,7,7,30] f32, target_tensor [32768,7,7,30] f32.
Output: np.ndarray shape (5,) f32 = (loss_xy, loss_wh, loss_obj, loss_noobj, loss_class).

Strategy: pure data parallel on batch dim; each core gets 4096 samples
(200704 cells). Host converts to fp16 and splits channels into fully
contiguous groups so the hot DVE ops coalesce into the 2x packed mode
(strided views of an interleaved [n,10] tile measure 1x or worse):
  - xy4  [n,4] cell-major (x0,y0,x1,y1)        both tensors
  - wh4  [n,4] cell-major (w0,h0,w1,h1)        both tensors
  - cf2  [n,2] cell-major (c0,c1)              both tensors
  - cls  [20,n] channel-major per chunk        both tensors
Per 392-cell chunk: IoU responsibility + five masked squared-diff partial
sums, fused on-chip. Weighted reductions run as premask-multiply (masks are
exactly 0/1) + in-place Square with accum_out on the scalar engine; the
class premask ANDs int32-reinterpreted fp16 pairs against a 0xFFFF mask;
reciprocal via the ~1cpe approx custom-DVE op. Each core returns a [128,20]
f32 partial-sum tile (5 losses x 4 chunks); host reduces and divides by N.
"""

import os
import sys

sys.path.insert(0, "/opt/trn_rl_repo")

import numpy as np

import concourse.bass as bass
import concourse.bacc as bacc
import concourse.tile as tile
from concourse import mybir
from concourse import bass_utils

F32 = mybir.dt.float32
F16 = mybir.dt.float16
I16 = mybir.dt.int16
I32 = mybir.dt.int32
ALU = mybir.AluOpType
ACT = mybir.ActivationFunctionType

S = 7
B = 2
C = 20
D = 30
N_FULL = 32768
N_CORES = 8
N_SHARD = N_FULL // N_CORES            # 4096 samples per core
R = N_SHARD * S * S                    # 200704 cells per core
P = 128                                # partitions
RP = R // P                            # 1568 cells per partition
NCK = 392                              # cells per partition per chunk
N_CH = RP // NCK                       # 4 chunks

PERM_XY = [0, 1, 5, 6]   # x0,y0,x1,y1
PERM_WH = [2, 3, 7, 8]   # w0,h0,w1,h1
PERM_CF = [4, 9]         # c0,c1


def _mk(ap, dims):
    """Rebuild the free dims of `ap` (keeping partition dim + offset) as
    `dims` = list of (step, count)."""
    new = [list(ap.ap[0])] + [[s, c] for s, c in dims]
    return bass.AP(tensor=ap.tensor, offset=ap.offset, ap=new)


def _ins(ap, pos, step, count):
    new = [list(x) for x in ap.ap]
    new.insert(pos, [step, count])
    return bass.AP(tensor=ap.tensor, offset=ap.offset, ap=new)


def build_program():
    nc = bacc.Bacc("TRN2", target_bir_lowering=False, debug=False)
    n = NCK

    def din(name, per_chunk):
        return nc.dram_tensor(name, [P, N_CH * per_chunk], F16, kind="ExternalInput")

    pbox, tbox = din("pbox", n * 10), din("tbox", n * 10)
    pcl, tcl = din("pcl", C * n), din("tcl", C * n)
    out = nc.dram_tensor("out", [P, 5 * N_CH], F32, kind="ExternalOutput")

    pbox_v = pbox.ap().rearrange("p (k a) -> p k a", k=N_CH, a=n * 10)
    tbox_v = tbox.ap().rearrange("p (k a) -> p k a", k=N_CH, a=n * 10)
    pcl_v = pcl.ap().rearrange("p (k c i) -> p k c i", k=N_CH, c=C, i=n)
    tcl_v = tcl.ap().rearrange("p (k c i) -> p k c i", k=N_CH, c=C, i=n)

    with tile.TileContext(nc) as tc:
        with (
            tc.tile_pool(name="raw", bufs=2) as raw,
            tc.tile_pool(name="tmp", bufs=1) as tmp,
            tc.tile_pool(name="persist", bufs=1) as persist,
        ):
            acc = persist.tile([P, 5 * N_CH], F32)

            for k in range(N_CH):
                # one block-major box DMA per tensor: [xy4(4n) | wh4(4n) | cf2(2n)]
                # contiguous inside the transfer, so every group view coalesces
                Bp = raw.tile([P, 10 * n], F16, tag="Bp")
                Bt = raw.tile([P, 10 * n], F16, tag="Bt")
                Pcl = raw.tile([P, C, n], F16, tag="Pcl")
                Tcl = raw.tile([P, C, n], F16, tag="Tcl")
                nc.sync.dma_start(out=Bp, in_=pbox_v[:, k])
                nc.sync.dma_start(out=Bt, in_=tbox_v[:, k])
                nc.sync.dma_start(out=Pcl, in_=pcl_v[:, k])
                nc.sync.dma_start(out=Tcl, in_=tcl_v[:, k])

                Pxy = Bp[:, 0:4 * n]            # flat [P,4n], cell-major (x0,y0,x1,y1)
                Pwh = Bp[:, 4 * n:8 * n]        # (w0,h0,w1,h1)
                Pcf = Bp[:, 8 * n:10 * n]       # (c0,c1)
                Txy = Bt[:, 0:4 * n]
                Twh = Bt[:, 4 * n:8 * n]
                Tcf = Bt[:, 8 * n:10 * n]
                obj_src = _mk(Bt[:, 8 * n], [(2, n)])   # target c0 per cell

                def sqacc(dm, col):
                    # in-place square: ACT streams read-then-write per element,
                    # so out == in is safe and avoids junk tiles whose reuse
                    # would couple engines across chunks
                    nc.scalar.activation(
                        dm, dm, ACT.Square,
                        accum_out=acc[:, 5 * k + col:5 * k + col + 1],
                    )

                def class_block():
                    # class (channel-major [P,20,n]): AND the fp16 diffs
                    # against a 0xFFFF/0x0000 obj mask through int32 views —
                    # pair-packing halves the cost vs a 1x broadcast multiply
                    ffi = tmp.tile([P, n], I16, tag="ffi")
                    nc.scalar.activation(ffi, obj_src, ACT.Copy, scale=-1.0)
                    ff32 = ffi.bitcast(I32)                       # [P, n/2]
                    ff32b = _mk(ff32[:, 0], [(0, C), (1, n // 2)])
                    dcl = tmp.tile([P, C, n], F16, tag="dcl")
                    dmcl = tmp.tile([P, C, n], F16, tag="dmcl")
                    nc.vector.tensor_tensor(dcl, Tcl, Pcl, op=ALU.subtract)
                    nc.vector.tensor_tensor(
                        dmcl.bitcast(I32), dcl.bitcast(I32), ff32b,
                        op=ALU.bitwise_and,
                    )
                    sqacc(dmcl, 4)

                # on the final chunk run the class loss first: its inputs are
                # prefetched by then, and the big ACT square-accum no longer
                # sits alone in the kernel tail
                cls_done = k == N_CH - 1
                if cls_done:
                    class_block()

                # ---- IoU stage (coords scaled x7: corners 3.5*wh -+ xy) ----
                t1 = tmp.tile([P, n, 4], F16, tag="t1")
                nc.vector.tensor_scalar(t1, Pwh, 3.5, None, op0=ALU.mult)
                nl4 = tmp.tile([P, n, 4], F16, tag="nl4")    # -(7l) both boxes
                r4 = tmp.tile([P, n, 4], F16, tag="r4")      # 7r both boxes
                nc.vector.tensor_tensor(nl4, t1, Pxy, op=ALU.subtract)
                nc.vector.tensor_tensor(r4, t1, Pxy, op=ALU.add)

                # target corners, box0 only
                txy0 = _mk(Bt[:, 0], [(4, n), (1, 2)])
                twh0 = _mk(Bt[:, 4 * n], [(4, n), (1, 2)])
                t2 = tmp.tile([P, n, 2], F16, tag="t2")
                nc.vector.tensor_scalar(t2, twh0, 3.5, None, op0=ALU.mult)
                nlt2 = tmp.tile([P, n, 2], F16, tag="nlt2")
                rt2 = tmp.tile([P, n, 2], F16, tag="rt2")
                nc.vector.tensor_tensor(nlt2, t2, txy0, op=ALU.subtract)
                nc.vector.tensor_tensor(rt2, t2, txy0, op=ALU.add)
                nlt2b = _mk(nlt2[:, 0, 0], [(2, n), (0, 2), (1, 2)])
                rt2b = _mk(rt2[:, 0, 0], [(2, n), (0, 2), (1, 2)])

                mln4 = tmp.tile([P, n, 4], F16, tag="mln4")
                mr4 = tmp.tile([P, n, 4], F16, tag="mr4")
                nc.vector.tensor_tensor(mln4, nl4, nlt2b, op=ALU.min)
                nc.vector.tensor_tensor(mr4, r4, rt2b, op=ALU.min)
                s4 = nl4  # dead, reuse
                nc.vector.tensor_tensor(s4, mln4, mr4, op=ALU.add)   # 7*(minr-maxl)
                cw4 = r4  # dead, reuse
                nc.vector.tensor_scalar(cw4, s4, 1.0 / 7.0, 0.0, op0=ALU.mult, op1=ALU.max)

                # per-box scalars, box-major [P,2,n]
                inter2 = tmp.tile([P, 2, n], F16, tag="inter2")
                areap2 = tmp.tile([P, 2, n], F16, tag="areap2")
                areat = tmp.tile([P, n], F16, tag="areat")
                cwx = _mk(cw4[:, 0, 0], [(2, 2), (4, n)])
                cwy = _mk(cw4[:, 0, 1], [(2, 2), (4, n)])
                nc.vector.tensor_tensor(inter2, cwx, cwy, op=ALU.mult)
                pw2 = _mk(Bp[:, 4 * n], [(2, 2), (4, n)])
                ph2 = _mk(Bp[:, 4 * n + 1], [(2, 2), (4, n)])
                nc.vector.tensor_tensor(areap2, pw2, ph2, op=ALU.mult)
                tw0 = _mk(Bt[:, 4 * n], [(4, n)])
                th0 = _mk(Bt[:, 4 * n + 1], [(4, n)])
                nc.vector.tensor_tensor(areat, tw0, th0, op=ALU.mult)

                u2h = tmp.tile([P, 2, n], F16, tag="u2h")
                u2 = tmp.tile([P, 2, n], F32, tag="u2")
                nc.vector.tensor_tensor(u2h, areap2, inter2, op=ALU.subtract)
                areatb = _ins(areat[:, :], 1, 0, 2)          # [box step0][cell step1]
                nc.vector.tensor_tensor(u2, u2h, areatb, op=ALU.add)

                rcp2 = tmp.tile([P, 2, n], F32, tag="rcp2")
                nc.vector.reciprocal_approx_fast(rcp2, u2)
                rcp16 = tmp.tile([P, 2, n], F16, tag="rcp16")
                nc.scalar.activation(rcp16, rcp2, ACT.Copy)
                iou2 = tmp.tile([P, 2, n], F16, tag="iou2")
                nc.vector.tensor_tensor(iou2, inter2, rcp16, op=ALU.mult)

                is1 = tmp.tile([P, n], F16, tag="is1")
                riou = tmp.tile([P, n], F16, tag="riou")
                nc.vector.tensor_tensor(is1, iou2[:, 1, :], iou2[:, 0, :], op=ALU.is_gt)
                nc.vector.tensor_tensor(riou, iou2[:, 1, :], iou2[:, 0, :], op=ALU.max)

                obj_c = tmp.tile([P, n], F16, tag="obj_c")
                nc.scalar.activation(obj_c, obj_src, ACT.Copy)

                resp = tmp.tile([P, 2, n], F16, tag="resp")
                nc.vector.tensor_tensor(resp[:, 1, :], obj_c, is1, op=ALU.mult)
                nc.vector.tensor_tensor(resp[:, 0, :], obj_c, resp[:, 1, :], op=ALU.subtract)

                # (r0,r0,r1,r1) per cell, compact so xy/wh premasks run at 2x
                resp4m = tmp.tile([P, n, 4], F16, tag="resp4m")
                resp4v = _mk(resp[:, 0, 0], [(1, n), (n, 2), (0, 2)])
                nc.scalar.activation(resp4m, resp4v, ACT.Copy)

                # ---- losses: premask (DVE) + in-place Square-with-accum (ACT)
                # xy
                dxy4 = tmp.tile([P, n, 4], F16, tag="dxy4")
                dm4 = tmp.tile([P, n, 4], F16, tag="dm4")
                nc.vector.tensor_tensor(dxy4, Txy, Pxy, op=ALU.subtract)
                nc.vector.tensor_tensor(dm4, dxy4, resp4m, op=ALU.mult)
                sqacc(dm4, 0)

                # wh (sqrt space)
                sp4 = tmp.tile([P, n, 4], F16, tag="sp4")
                st4 = tmp.tile([P, n, 4], F16, tag="st4")
                nc.scalar.activation(sp4, Pwh, ACT.Sqrt)
                nc.scalar.activation(st4, Twh, ACT.Sqrt)
                # dxy4's last reader is the dm4 multiply (DVE) — same-engine reuse
                dwh4 = dxy4
                dmw4 = mr4  # dead, reuse
                nc.vector.tensor_tensor(dwh4, st4, sp4, op=ALU.subtract)
                nc.vector.tensor_tensor(dmw4, dwh4, resp4m, op=ALU.mult)
                sqacc(dmw4, 1)

                # obj conf vs responsible-iou, box-major [P,2,n]: diff is 1x
                # (broadcast riou), premask is all-compact 2x against resp
                dc2 = tmp.tile([P, 2, n], F16, tag="dc2")
                dmc2 = tmp.tile([P, 2, n], F16, tag="dmc2")
                rioub = _ins(riou[:, :], 1, 0, 2)                 # [boxdup][cell]
                pc2bm = _mk(Bp[:, 8 * n], [(1, 2), (2, n)])       # [box][cell]
                nc.vector.tensor_tensor(dc2, rioub, pc2bm, op=ALU.subtract)
                nc.vector.tensor_tensor(dmc2, dc2, resp, op=ALU.mult)
                sqacc(dmc2, 2)

                # noobj conf: noobj*(tc-pc)^2 == ((tc*pc)-pc)^2 since tc in {0,1}
                m2 = tmp.tile([P, n, 2], F16, tag="m2")
                dmn2 = tmp.tile([P, n, 2], F16, tag="dmn2")
                nc.vector.tensor_tensor(m2, Tcf, Pcf, op=ALU.mult)
                nc.vector.tensor_tensor(dmn2, m2, Pcf, op=ALU.subtract)
                sqacc(dmn2, 3)

                if not cls_done:
                    class_block()

            nc.sync.dma_start(out=out.ap(), in_=acc)

    nc.compile()
    return nc


_nc_cache = None
LAST_EXEC_NS = None
LAST_RESULT = None


def _get_nc():
    global _nc_cache
    if _nc_cache is None:
        _nc_cache = build_program()
    return _nc_cache


def _prep(full):
    """[N*S*S, 30] f32 -> per-core fp16 (box blocks [k][xy4|wh4|cf2], cls)."""
    A = np.asarray(full, dtype=np.float32).reshape(N_CORES, P, N_CH, NCK, D)
    A16 = A.astype(np.float16)
    xy = A16[..., PERM_XY].reshape(N_CORES, P, N_CH, -1)
    wh = A16[..., PERM_WH].reshape(N_CORES, P, N_CH, -1)
    cf = A16[..., PERM_CF].reshape(N_CORES, P, N_CH, -1)
    box = np.ascontiguousarray(np.concatenate([xy, wh, cf], axis=-1)).reshape(
        N_CORES, P, -1
    )
    cl = np.ascontiguousarray(A16[..., 10:30].transpose(0, 1, 2, 4, 3)).reshape(
        N_CORES, P, -1
    )
    return box, cl


def kernel(pred_tensor, target_tensor):
    global LAST_EXEC_NS, LAST_RESULT
    pred = np.asarray(pred_tensor).reshape(N_FULL * S * S, D)
    tgt = np.asarray(target_tensor).reshape(N_FULL * S * S, D)

    pb, pc = _prep(pred)
    tb, tc = _prep(tgt)

    in_maps = []
    for i in range(N_CORES):
        in_maps.append({"pbox": pb[i], "tbox": tb[i], "pcl": pc[i], "tcl": tc[i]})

    nc = _get_nc()
    trace = bool(os.environ.get("KERNEL_TRACE"))
    tmpdir = os.environ.get("KERNEL_TRACE_DIR") or None
    res = bass_utils.run_bass_kernel_spmd(
        nc, in_maps, core_ids=list(range(N_CORES)), trace=trace, tmpdir=tmpdir
    )
    LAST_RESULT = res
    if res.exec_time_ns is not None:
        LAST_EXEC_NS = res.exec_time_ns
    total = np.zeros(5, dtype=np.float64)
    for m in res.results:
        total += m["out"].astype(np.float64).sum(axis=0).reshape(N_CH, 5).sum(axis=0)
    losses = (total / float(N_FULL)).astype(np.float32)
    return losses


# revision 21
# speedup vs baseline: 1.1750x; 1.0291x over previous
"""YOLO-style loss kernel for Trainium2, SPMD over 8 NeuronCores.

Inputs (full): pred_tensor [32768,7,7,30] f32, target_tensor [32768,7,7,30] f32.
Output: np.ndarray shape (5,) f32 = (loss_xy, loss_wh, loss_obj, loss_noobj, loss_class).

Strategy: pure data parallel on batch dim; each core gets 4096 samples
(200704 cells). Host converts to fp16 and splits channels into fully
contiguous groups so the hot DVE ops coalesce into the 2x packed mode
(strided views of an interleaved [n,10] tile measure 1x or worse):
  - xy4  [n,4] cell-major (x0,y0,x1,y1)        both tensors
  - wh4  [n,4] cell-major (w0,h0,w1,h1)        both tensors
  - cf2  [n,2] cell-major (c0,c1)              both tensors
  - cls  [20,n] channel-major per chunk        both tensors
Per 392-cell chunk: IoU responsibility + five masked squared-diff partial
sums, fused on-chip. Weighted reductions run as premask-multiply (masks are
exactly 0/1) + in-place Square with accum_out on the scalar engine; the
class premask ANDs int32-reinterpreted fp16 pairs against a 0xFFFF mask;
reciprocal via the ~1cpe approx custom-DVE op. Each core returns a [128,20]
f32 partial-sum tile (5 losses x 4 chunks); host reduces and divides by N.
"""

import os
import sys

sys.path.insert(0, "/opt/trn_rl_repo")

import numpy as np

import concourse.bass as bass
import concourse.bacc as bacc
import concourse.tile as tile
from concourse import mybir
from concourse import bass_utils

F32 = mybir.dt.float32
F16 = mybir.dt.float16
I16 = mybir.dt.int16
I32 = mybir.dt.int32
ALU = mybir.AluOpType
ACT = mybir.ActivationFunctionType

S = 7
B = 2
C = 20
D = 30
N_FULL = 32768
N_CORES = 8
N_SHARD = N_FULL // N_CORES            # 4096 samples per core
R = N_SHARD * S * S                    # 200704 cells per core
P = 128                                # partitions
RP = R // P                            # 1568 cells per partition
NCK = 392                              # cells per partition per chunk
N_CH = RP // NCK                       # 4 chunks

PERM_XY = [0, 1, 5, 6]   # x0,y0,x1,y1
PERM_WH = [2, 3, 7, 8]   # w0,h0,w1,h1
PERM_CF = [4, 9]         # c0,c1


def _mk(ap, dims):
    """Rebuild the free dims of `ap` (keeping partition dim + offset) as
    `dims` = list of (step, count)."""
    new = [list(ap.ap[0])] + [[s, c] for s, c in dims]
    return bass.AP(tensor=ap.tensor, offset=ap.offset, ap=new)


def _ins(ap, pos, step, count):
    new = [list(x) for x in ap.ap]
    new.insert(pos, [step, count])
    return bass.AP(tensor=ap.tensor, offset=ap.offset, ap=new)


def build_program():
    nc = bacc.Bacc("TRN2", target_bir_lowering=False, debug=False)
    n = NCK

    def din(name, per_chunk):
        return nc.dram_tensor(name, [P, N_CH * per_chunk], F16, kind="ExternalInput")

    pbox, tbox = din("pbox", n * 10), din("tbox", n * 10)
    pcl, tcl = din("pcl", C * n), din("tcl", C * n)
    out = nc.dram_tensor("out", [P, 5 * N_CH], F32, kind="ExternalOutput")

    pbox_v = pbox.ap().rearrange("p (k a) -> p k a", k=N_CH, a=n * 10)
    tbox_v = tbox.ap().rearrange("p (k a) -> p k a", k=N_CH, a=n * 10)
    pcl_v = pcl.ap().rearrange("p (k c i) -> p k c i", k=N_CH, c=C, i=n)
    tcl_v = tcl.ap().rearrange("p (k c i) -> p k c i", k=N_CH, c=C, i=n)

    with tile.TileContext(nc) as tc:
        with (
            tc.tile_pool(name="raw", bufs=2) as raw,
            tc.tile_pool(name="tmp", bufs=1) as tmp,
            tc.tile_pool(name="persist", bufs=1) as persist,
        ):
            acc = persist.tile([P, 5 * N_CH], F32)

            for k in range(N_CH):
                # one block-major box DMA per tensor: [xy4(4n) | wh4(4n) | cf2(2n)]
                # contiguous inside the transfer, so every group view coalesces
                Bp = raw.tile([P, 10 * n], F16, tag="Bp")
                Bt = raw.tile([P, 10 * n], F16, tag="Bt")
                Pcl = raw.tile([P, C, n], F16, tag="Pcl")
                Tcl = raw.tile([P, C, n], F16, tag="Tcl")
                nc.sync.dma_start(out=Bp, in_=pbox_v[:, k])
                nc.sync.dma_start(out=Bt, in_=tbox_v[:, k])
                nc.sync.dma_start(out=Pcl, in_=pcl_v[:, k])
                nc.sync.dma_start(out=Tcl, in_=tcl_v[:, k])

                # box-major rows: x0,y0,x1,y1 | w0,h0,w1,h1 | c0,c1, each a
                # contiguous n-row, so every group op fully coalesces
                Pxy = Bp[:, 0:4 * n]
                Pwh = Bp[:, 4 * n:8 * n]
                Pcf = Bp[:, 8 * n:10 * n]
                Txy = Bt[:, 0:4 * n]
                Twh = Bt[:, 4 * n:8 * n]
                Tcf = Bt[:, 8 * n:10 * n]
                obj_src = Bt[:, 8 * n:9 * n]    # target c0 row, compact [P,n]

                def sqacc(dm, col):
                    # in-place square: ACT streams read-then-write per element,
                    # so out == in is safe and avoids junk tiles whose reuse
                    # would couple engines across chunks
                    nc.scalar.activation(
                        dm, dm, ACT.Square,
                        accum_out=acc[:, 5 * k + col:5 * k + col + 1],
                    )

                def class_block():
                    # class (channel-major [P,20,n]): AND the fp16 diffs
                    # against a 0xFFFF/0x0000 obj mask through int32 views —
                    # pair-packing halves the cost vs a 1x broadcast multiply
                    ffi = tmp.tile([P, n], I16, tag="ffi")
                    nc.scalar.activation(ffi, obj_src, ACT.Copy, scale=-1.0)
                    ff32 = ffi.bitcast(I32)                       # [P, n/2]
                    ff32b = _mk(ff32[:, 0], [(0, C), (1, n // 2)])
                    dcl = tmp.tile([P, C, n], F16, tag="dcl")
                    dmcl = tmp.tile([P, C, n], F16, tag="dmcl")
                    nc.vector.tensor_tensor(dcl, Tcl, Pcl, op=ALU.subtract)
                    nc.vector.tensor_tensor(
                        dmcl.bitcast(I32), dcl.bitcast(I32), ff32b,
                        op=ALU.bitwise_and,
                    )
                    sqacc(dmcl, 4)

                # on the final chunk run the class loss first: its inputs are
                # prefetched by then, and the big ACT square-accum no longer
                # sits alone in the kernel tail
                cls_done = k == N_CH - 1
                if cls_done:
                    class_block()

                # ---- IoU stage (coords scaled x7: corners 3.5*wh -+ xy) ----
                t1 = tmp.tile([P, 4, n], F16, tag="t1")
                nc.vector.tensor_scalar(t1, Pwh, 3.5, None, op0=ALU.mult)
                nl4 = tmp.tile([P, 4, n], F16, tag="nl4")    # -(7l) both boxes
                r4 = tmp.tile([P, 4, n], F16, tag="r4")      # 7r both boxes
                nc.vector.tensor_tensor(nl4, t1, Pxy, op=ALU.subtract)
                nc.vector.tensor_tensor(r4, t1, Pxy, op=ALU.add)

                # target corners, box0 only (x0,y0 / w0,h0 rows contiguous)
                txy0 = Bt[:, 0:2 * n]
                twh0 = Bt[:, 4 * n:6 * n]
                t2 = tmp.tile([P, 2, n], F16, tag="t2")
                nc.vector.tensor_scalar(t2, twh0, 3.5, None, op0=ALU.mult)
                nlt2 = tmp.tile([P, 2, n], F16, tag="nlt2")
                rt2 = tmp.tile([P, 2, n], F16, tag="rt2")
                nc.vector.tensor_tensor(nlt2, t2, txy0, op=ALU.subtract)
                nc.vector.tensor_tensor(rt2, t2, txy0, op=ALU.add)
                # rows (x,y,x,y): [boxdup step0][coord-row step n][cell step1]
                nlt2b = _mk(nlt2[:, 0, 0], [(0, 2), (n, 2), (1, n)])
                rt2b = _mk(rt2[:, 0, 0], [(0, 2), (n, 2), (1, n)])

                mln4 = tmp.tile([P, 4, n], F16, tag="mln4")
                mr4 = tmp.tile([P, 4, n], F16, tag="mr4")
                nc.vector.tensor_tensor(mln4, nl4, nlt2b, op=ALU.min)
                nc.vector.tensor_tensor(mr4, r4, rt2b, op=ALU.min)
                s4 = nl4  # dead, reuse
                nc.vector.tensor_tensor(s4, mln4, mr4, op=ALU.add)   # 7*(minr-maxl)
                cw4 = r4  # dead, reuse
                nc.vector.tensor_scalar(cw4, s4, 1.0 / 7.0, 0.0, op0=ALU.mult, op1=ALU.max)

                # per-box scalars, box-major [P,2,n]
                inter2 = tmp.tile([P, 2, n], F16, tag="inter2")
                areap2 = tmp.tile([P, 2, n], F16, tag="areap2")
                areat = tmp.tile([P, n], F16, tag="areat")
                cwx = cw4[:, 0:4:2, :]                           # x rows {0,2}
                cwy = cw4[:, 1:4:2, :]                           # y rows {1,3}
                nc.vector.tensor_tensor(inter2, cwx, cwy, op=ALU.mult)
                pw2 = _mk(Bp[:, 4 * n], [(2 * n, 2), (1, n)])    # w rows {0,2}
                ph2 = _mk(Bp[:, 5 * n], [(2 * n, 2), (1, n)])    # h rows {1,3}
                nc.vector.tensor_tensor(areap2, pw2, ph2, op=ALU.mult)
                nc.vector.tensor_tensor(areat, Bt[:, 4 * n:5 * n], Bt[:, 5 * n:6 * n], op=ALU.mult)

                u2h = tmp.tile([P, 2, n], F16, tag="u2h")
                u2 = tmp.tile([P, 2, n], F32, tag="u2")
                nc.vector.tensor_tensor(u2h, areap2, inter2, op=ALU.subtract)
                areatb = _ins(areat[:, :], 1, 0, 2)          # [box step0][cell step1]
                nc.vector.tensor_tensor(u2, u2h, areatb, op=ALU.add)

                rcp2 = tmp.tile([P, 2, n], F32, tag="rcp2")
                nc.vector.reciprocal_approx_fast(rcp2, u2)
                rcp16 = tmp.tile([P, 2, n], F16, tag="rcp16")
                nc.scalar.activation(rcp16, rcp2, ACT.Copy)
                iou2 = tmp.tile([P, 2, n], F16, tag="iou2")
                nc.vector.tensor_tensor(iou2, inter2, rcp16, op=ALU.mult)

                is1 = tmp.tile([P, n], F16, tag="is1")
                riou = tmp.tile([P, n], F16, tag="riou")
                nc.vector.tensor_tensor(is1, iou2[:, 1, :], iou2[:, 0, :], op=ALU.is_gt)
                nc.vector.tensor_tensor(riou, iou2[:, 1, :], iou2[:, 0, :], op=ALU.max)

                resp = tmp.tile([P, 2, n], F16, tag="resp")
                nc.vector.tensor_tensor(resp[:, 1, :], obj_src, is1, op=ALU.mult)
                nc.vector.tensor_tensor(resp[:, 0, :], obj_src, resp[:, 1, :], op=ALU.subtract)

                # rows (r0,r0,r1,r1), compact so xy/wh premasks run at 2x
                resp4m = tmp.tile([P, 4, n], F16, tag="resp4m")
                resp4v = _mk(resp[:, 0, 0], [(n, 2), (0, 2), (1, n)])
                nc.scalar.activation(resp4m, resp4v, ACT.Copy)

                # ---- losses: premask (DVE) + in-place Square-with-accum (ACT)
                # xy
                dxy4 = tmp.tile([P, 4, n], F16, tag="dxy4")
                dm4 = tmp.tile([P, 4, n], F16, tag="dm4")
                nc.vector.tensor_tensor(dxy4, Txy, Pxy, op=ALU.subtract)
                nc.vector.tensor_tensor(dm4, dxy4, resp4m, op=ALU.mult)
                sqacc(dm4, 0)

                # wh (sqrt space)
                sp4 = tmp.tile([P, 4, n], F16, tag="sp4")
                st4 = tmp.tile([P, 4, n], F16, tag="st4")
                nc.scalar.activation(sp4, Pwh, ACT.Sqrt)
                nc.scalar.activation(st4, Twh, ACT.Sqrt)
                # dxy4's last reader is the dm4 multiply (DVE) — same-engine reuse
                dwh4 = dxy4
                dmw4 = mr4  # dead, reuse
                nc.vector.tensor_tensor(dwh4, st4, sp4, op=ALU.subtract)
                nc.vector.tensor_tensor(dmw4, dwh4, resp4m, op=ALU.mult)
                sqacc(dmw4, 1)

                # obj conf vs responsible-iou, box-major [P,2,n]: conf rows
                # are compact so diff and premask both pack
                dc2 = tmp.tile([P, 2, n], F16, tag="dc2")
                dmc2 = tmp.tile([P, 2, n], F16, tag="dmc2")
                rioub = _ins(riou[:, :], 1, 0, 2)                 # [boxdup][cell]
                nc.vector.tensor_tensor(dc2, rioub, Pcf, op=ALU.subtract)
                nc.vector.tensor_tensor(dmc2, dc2, resp, op=ALU.mult)
                sqacc(dmc2, 2)

                # noobj conf: noobj*(tc-pc)^2 == ((tc*pc)-pc)^2 since tc in {0,1}
                m2 = tmp.tile([P, 2, n], F16, tag="m2")
                dmn2 = tmp.tile([P, 2, n], F16, tag="dmn2")
                nc.vector.tensor_tensor(m2, Tcf, Pcf, op=ALU.mult)
                nc.vector.tensor_tensor(dmn2, m2, Pcf, op=ALU.subtract)
                sqacc(dmn2, 3)

                if not cls_done:
                    class_block()

            nc.sync.dma_start(out=out.ap(), in_=acc)

    nc.compile()
    return nc


_nc_cache = None
LAST_EXEC_NS = None
LAST_RESULT = None


def _get_nc():
    global _nc_cache
    if _nc_cache is None:
        _nc_cache = build_program()
    return _nc_cache


def _prep(full):
    """[N*S*S, 30] f32 -> per-core fp16 (box blocks [k][xy4|wh4|cf2], cls)."""
    A = np.asarray(full, dtype=np.float32).reshape(N_CORES, P, N_CH, NCK, D)
    A16 = A.astype(np.float16)
    # box-major rows: per chunk [x0,y0,x1,y1 | w0,h0,w1,h1 | c0,c1], each row
    # a contiguous n-vector
    xy = A16[..., PERM_XY].transpose(0, 1, 2, 4, 3)
    wh = A16[..., PERM_WH].transpose(0, 1, 2, 4, 3)
    cf = A16[..., PERM_CF].transpose(0, 1, 2, 4, 3)
    box = np.ascontiguousarray(np.concatenate([xy, wh, cf], axis=-2)).reshape(
        N_CORES, P, -1
    )
    cl = np.ascontiguousarray(A16[..., 10:30].transpose(0, 1, 2, 4, 3)).reshape(
        N_CORES, P, -1
    )
    return box, cl


def kernel(pred_tensor, target_tensor):
    global LAST_EXEC_NS, LAST_RESULT
    pred = np.asarray(pred_tensor).reshape(N_FULL * S * S, D)
    tgt = np.asarray(target_tensor).reshape(N_FULL * S * S, D)

    pb, pc = _prep(pred)
    tb, tc = _prep(tgt)

    in_maps = []
    for i in range(N_CORES):
        in_maps.append({"pbox": pb[i], "tbox": tb[i], "pcl": pc[i], "tcl": tc[i]})

    nc = _get_nc()
    trace = bool(os.environ.get("KERNEL_TRACE"))
    tmpdir = os.environ.get("KERNEL_TRACE_DIR") or None
    res = bass_utils.run_bass_kernel_spmd(
        nc, in_maps, core_ids=list(range(N_CORES)), trace=trace, tmpdir=tmpdir
    )
    LAST_RESULT = res
    if res.exec_time_ns is not None:
        LAST_EXEC_NS = res.exec_time_ns
    total = np.zeros(5, dtype=np.float64)
    for m in res.results:
        total += m["out"].astype(np.float64).sum(axis=0).reshape(N_CH, 5).sum(axis=0)
    losses = (total / float(N_FULL)).astype(np.float32)
    return losses


# revision 23
# speedup vs baseline: 1.2429x; 1.0578x over previous
"""YOLO-style loss kernel for Trainium2, SPMD over 8 NeuronCores.

Inputs (full): pred_tensor [32768,7,7,30] f32, target_tensor [32768,7,7,30] f32.
Output: np.ndarray shape (5,) f32 = (loss_xy, loss_wh, loss_obj, loss_noobj, loss_class).

Strategy: pure data parallel on batch dim; each core gets 4096 samples
(200704 cells). Host converts to fp16 and splits channels into fully
contiguous groups so the hot DVE ops coalesce into the 2x packed mode
(strided views of an interleaved [n,10] tile measure 1x or worse):
  - xy4  [n,4] cell-major (x0,y0,x1,y1)        both tensors
  - wh4  [n,4] cell-major (w0,h0,w1,h1)        both tensors
  - cf2  [n,2] cell-major (c0,c1)              both tensors
  - cls  [20,n] channel-major per chunk        both tensors
Per 392-cell chunk: IoU responsibility + five masked squared-diff partial
sums, fused on-chip. Weighted reductions run as premask-multiply (masks are
exactly 0/1) + in-place Square with accum_out on the scalar engine; the
class premask ANDs int32-reinterpreted fp16 pairs against a 0xFFFF mask;
reciprocal via the ~1cpe approx custom-DVE op. Each core returns a [128,20]
f32 partial-sum tile (5 losses x 4 chunks); host reduces and divides by N.
"""

import os
import sys

sys.path.insert(0, "/opt/trn_rl_repo")

import numpy as np

import concourse.bass as bass
import concourse.bacc as bacc
import concourse.tile as tile
from concourse import mybir
from concourse import bass_utils

F32 = mybir.dt.float32
F16 = mybir.dt.float16
I16 = mybir.dt.int16
I32 = mybir.dt.int32
ALU = mybir.AluOpType
ACT = mybir.ActivationFunctionType

S = 7
B = 2
C = 20
D = 30
N_FULL = 32768
N_CORES = 8
N_SHARD = N_FULL // N_CORES            # 4096 samples per core
R = N_SHARD * S * S                    # 200704 cells per core
P = 128                                # partitions
RP = R // P                            # 1568 cells per partition
NCK = 392                              # cells per partition per chunk
N_CH = RP // NCK                       # 4 chunks

PERM_XY = [0, 1, 5, 6]   # x0,y0,x1,y1
PERM_WH = [2, 3, 7, 8]   # w0,h0,w1,h1
PERM_CF = [4, 9]         # c0,c1


def _mk(ap, dims):
    """Rebuild the free dims of `ap` (keeping partition dim + offset) as
    `dims` = list of (step, count)."""
    new = [list(ap.ap[0])] + [[s, c] for s, c in dims]
    return bass.AP(tensor=ap.tensor, offset=ap.offset, ap=new)


def _ins(ap, pos, step, count):
    new = [list(x) for x in ap.ap]
    new.insert(pos, [step, count])
    return bass.AP(tensor=ap.tensor, offset=ap.offset, ap=new)


def build_program():
    nc = bacc.Bacc("TRN2", target_bir_lowering=False, debug=False)
    n = NCK

    def din(name, per_chunk):
        return nc.dram_tensor(name, [P, N_CH * per_chunk], F16, kind="ExternalInput")

    pbox, tbox = din("pbox", n * 10), din("tbox", n * 10)
    pcl, tcl = din("pcl", C * n), din("tcl", C * n)
    out = nc.dram_tensor("out", [P, 5 * N_CH], F32, kind="ExternalOutput")

    pbox_v = pbox.ap().rearrange("p (k a) -> p k a", k=N_CH, a=n * 10)
    tbox_v = tbox.ap().rearrange("p (k a) -> p k a", k=N_CH, a=n * 10)
    pcl_v = pcl.ap().rearrange("p (k c i) -> p k c i", k=N_CH, c=C, i=n)
    tcl_v = tcl.ap().rearrange("p (k c i) -> p k c i", k=N_CH, c=C, i=n)

    with tile.TileContext(nc) as tc:
        with (
            tc.tile_pool(name="raw", bufs=2) as raw,
            tc.tile_pool(name="tmp", bufs=1) as tmp,
            tc.tile_pool(name="persist", bufs=1) as persist,
        ):
            acc = persist.tile([P, 5 * N_CH], F32)

            for k in range(N_CH):
                # one block-major box DMA per tensor: [xy4(4n) | wh4(4n) | cf2(2n)]
                # contiguous inside the transfer, so every group view coalesces
                Bp = raw.tile([P, 10 * n], F16, tag="Bp")
                Bt = raw.tile([P, 10 * n], F16, tag="Bt")
                Pcl = raw.tile([P, C, n], F16, tag="Pcl")
                Tcl = raw.tile([P, C, n], F16, tag="Tcl")
                nc.sync.dma_start(out=Bp, in_=pbox_v[:, k])
                nc.sync.dma_start(out=Bt, in_=tbox_v[:, k])
                nc.sync.dma_start(out=Pcl, in_=pcl_v[:, k])
                nc.sync.dma_start(out=Tcl, in_=tcl_v[:, k])

                # box-major rows: x0,y0,x1,y1 | w0,h0,w1,h1 | c0,c1, each a
                # contiguous n-row, so every group op fully coalesces
                Pxy = Bp[:, 0:4 * n]
                Pwh = Bp[:, 4 * n:8 * n]
                Pcf = Bp[:, 8 * n:10 * n]
                Txy = Bt[:, 0:4 * n]
                Twh = Bt[:, 4 * n:8 * n]
                Tcf = Bt[:, 8 * n:10 * n]
                obj_src = Bt[:, 8 * n:9 * n]    # target c0 row, compact [P,n]

                def sqacc(dm, col):
                    # in-place square: ACT streams read-then-write per element,
                    # so out == in is safe and avoids junk tiles whose reuse
                    # would couple engines across chunks
                    nc.scalar.activation(
                        dm, dm, ACT.Square,
                        accum_out=acc[:, 5 * k + col:5 * k + col + 1],
                    )

                def class_block():
                    # class (channel-major [P,20,n]): AND the fp16 diffs
                    # against a 0xFFFF/0x0000 obj mask through int32 views —
                    # pair-packing halves the cost vs a 1x broadcast multiply
                    ffi = tmp.tile([P, n], I16, tag="ffi")
                    nc.scalar.activation(ffi, obj_src, ACT.Copy, scale=-1.0)
                    ff32 = ffi.bitcast(I32)                       # [P, n/2]
                    ff32b = _mk(ff32[:, 0], [(0, C), (1, n // 2)])
                    dcl = tmp.tile([P, C, n], F16, tag="dcl")
                    dmcl = tmp.tile([P, C, n], F16, tag="dmcl")
                    nc.vector.tensor_tensor(dcl, Tcl, Pcl, op=ALU.subtract)
                    nc.vector.tensor_tensor(
                        dmcl.bitcast(I32), dcl.bitcast(I32), ff32b,
                        op=ALU.bitwise_and,
                    )
                    sqacc(dmcl, 4)

                # on the final chunk run the class loss first: its inputs are
                # prefetched by then, and the big ACT square-accum no longer
                # sits alone in the kernel tail
                cls_done = k == N_CH - 1
                if cls_done:
                    class_block()

                # ---- IoU stage (coords scaled x7: corners 3.5*wh -+ xy) ----
                t1 = tmp.tile([P, 4, n], F16, tag="t1")
                nc.vector.tensor_scalar(t1, Pwh, 3.5, None, op0=ALU.mult)
                nl4 = tmp.tile([P, 4, n], F16, tag="nl4")    # -(7l) both boxes
                r4 = tmp.tile([P, 4, n], F16, tag="r4")      # 7r both boxes
                nc.vector.tensor_tensor(nl4, t1, Pxy, op=ALU.subtract)
                nc.vector.tensor_tensor(r4, t1, Pxy, op=ALU.add)

                # target corners, box0 only (x0,y0 / w0,h0 rows contiguous)
                txy0 = Bt[:, 0:2 * n]
                twh0 = Bt[:, 4 * n:6 * n]
                t2 = tmp.tile([P, 2, n], F16, tag="t2")
                nc.vector.tensor_scalar(t2, twh0, 3.5, None, op0=ALU.mult)
                nlt2 = tmp.tile([P, 2, n], F16, tag="nlt2")
                rt2 = tmp.tile([P, 2, n], F16, tag="rt2")
                nc.vector.tensor_tensor(nlt2, t2, txy0, op=ALU.subtract)
                nc.vector.tensor_tensor(rt2, t2, txy0, op=ALU.add)
                # rows (x,y,x,y): [boxdup step0][coord-row step n][cell step1]
                nlt2b = _mk(nlt2[:, 0, 0], [(0, 2), (n, 2), (1, n)])
                rt2b = _mk(rt2[:, 0, 0], [(0, 2), (n, 2), (1, n)])

                mln4 = tmp.tile([P, 4, n], F16, tag="mln4")
                mr4 = tmp.tile([P, 4, n], F16, tag="mr4")
                nc.vector.tensor_tensor(mln4, nl4, nlt2b, op=ALU.min)
                nc.vector.tensor_tensor(mr4, r4, rt2b, op=ALU.min)
                s4 = nl4  # dead, reuse
                nc.vector.tensor_tensor(s4, mln4, mr4, op=ALU.add)   # 7*(minr-maxl)
                cw4 = r4  # dead, reuse
                nc.vector.tensor_scalar(cw4, s4, 1.0 / 7.0, 0.0, op0=ALU.mult, op1=ALU.max)

                # per-box scalars, box-major [P,2,n]
                inter2 = tmp.tile([P, 2, n], F16, tag="inter2")
                areap2 = tmp.tile([P, 2, n], F16, tag="areap2")
                areat = tmp.tile([P, n], F16, tag="areat")
                cwx = cw4[:, 0:4:2, :]                           # x rows {0,2}
                cwy = cw4[:, 1:4:2, :]                           # y rows {1,3}
                nc.vector.tensor_tensor(inter2, cwx, cwy, op=ALU.mult)
                pw2 = _mk(Bp[:, 4 * n], [(2 * n, 2), (1, n)])    # w rows {0,2}
                ph2 = _mk(Bp[:, 5 * n], [(2 * n, 2), (1, n)])    # h rows {1,3}
                nc.vector.tensor_tensor(areap2, pw2, ph2, op=ALU.mult)
                nc.vector.tensor_tensor(areat, Bt[:, 4 * n:5 * n], Bt[:, 5 * n:6 * n], op=ALU.mult)

                u2h = tmp.tile([P, 2, n], F16, tag="u2h")
                u2 = tmp.tile([P, 2, n], F32, tag="u2")
                nc.vector.tensor_tensor(u2h, areap2, inter2, op=ALU.subtract)
                areatb = _ins(areat[:, :], 1, 0, 2)          # [box step0][cell step1]
                nc.vector.tensor_tensor(u2, u2h, areatb, op=ALU.add)

                rcp2 = tmp.tile([P, 2, n], F32, tag="rcp2")
                nc.vector.reciprocal_approx_fast(rcp2, u2)
                rcp16 = tmp.tile([P, 2, n], F16, tag="rcp16")
                nc.scalar.activation(rcp16, rcp2, ACT.Copy)
                iou2 = tmp.tile([P, 2, n], F16, tag="iou2")
                nc.vector.tensor_tensor(iou2, inter2, rcp16, op=ALU.mult)

                is1 = tmp.tile([P, n], F16, tag="is1")
                riou = tmp.tile([P, n], F16, tag="riou")
                nc.vector.tensor_tensor(is1, iou2[:, 1, :], iou2[:, 0, :], op=ALU.is_gt)
                nc.vector.tensor_tensor(riou, iou2[:, 1, :], iou2[:, 0, :], op=ALU.max)

                resp = tmp.tile([P, 2, n], F16, tag="resp")
                nc.vector.tensor_tensor(resp[:, 1, :], obj_src, is1, op=ALU.mult)
                nc.vector.tensor_tensor(resp[:, 0, :], obj_src, resp[:, 1, :], op=ALU.subtract)

                # ---- losses: premask (DVE) + in-place Square-with-accum (ACT)
                # xy and wh diffs share one [P,8,n] tile; a single broadcast
                # premask multiply covers both (broadcast dims do not break
                # the 2x packed mode), then two sqaccs split the accum columns
                d8 = tmp.tile([P, 8, n], F16, tag="d8")
                dm8 = tmp.tile([P, 8, n], F16, tag="dm8")
                nc.vector.tensor_tensor(d8[:, 0:4, :], Txy, Pxy, op=ALU.subtract)
                sp4 = tmp.tile([P, 4, n], F16, tag="sp4")
                st4 = tmp.tile([P, 4, n], F16, tag="st4")
                nc.scalar.activation(sp4, Pwh, ACT.Sqrt)
                nc.scalar.activation(st4, Twh, ACT.Sqrt)
                nc.vector.tensor_tensor(d8[:, 4:8, :], st4, sp4, op=ALU.subtract)
                # rows (r0,r0,r1,r1): [box n][coorddup 0][cell 1] — 3 free dims
                # (4 exceeds the ISA AP limit); broadcast does not break 2x
                resp4b = _mk(resp[:, 0, 0], [(n, 2), (0, 2), (1, n)])
                nc.vector.tensor_tensor(dm8[:, 0:4, :], d8[:, 0:4, :], resp4b, op=ALU.mult)
                nc.vector.tensor_tensor(dm8[:, 4:8, :], d8[:, 4:8, :], resp4b, op=ALU.mult)
                sqacc(dm8[:, 0:4, :], 0)
                sqacc(dm8[:, 4:8, :], 1)

                # obj conf vs responsible-iou, box-major [P,2,n]: conf rows
                # are compact so diff and premask both pack
                dc2 = tmp.tile([P, 2, n], F16, tag="dc2")
                dmc2 = tmp.tile([P, 2, n], F16, tag="dmc2")
                rioub = _ins(riou[:, :], 1, 0, 2)                 # [boxdup][cell]
                nc.vector.tensor_tensor(dc2, rioub, Pcf, op=ALU.subtract)
                nc.vector.tensor_tensor(dmc2, dc2, resp, op=ALU.mult)
                sqacc(dmc2, 2)

                # noobj conf: noobj*(tc-pc)^2 == ((tc*pc)-pc)^2 since tc in {0,1}
                m2 = tmp.tile([P, 2, n], F16, tag="m2")
                dmn2 = tmp.tile([P, 2, n], F16, tag="dmn2")
                nc.vector.tensor_tensor(m2, Tcf, Pcf, op=ALU.mult)
                nc.vector.tensor_tensor(dmn2, m2, Pcf, op=ALU.subtract)
                sqacc(dmn2, 3)

                if not cls_done:
                    class_block()

            nc.sync.dma_start(out=out.ap(), in_=acc)

    nc.compile()
    return nc


_nc_cache = None
LAST_EXEC_NS = None
LAST_RESULT = None


def _get_nc():
    global _nc_cache
    if _nc_cache is None:
        _nc_cache = build_program()
    return _nc_cache


def _prep(full):
    """[N*S*S, 30] f32 -> per-core fp16 (box blocks [k][xy4|wh4|cf2], cls)."""
    A = np.asarray(full, dtype=np.float32).reshape(N_CORES, P, N_CH, NCK, D)
    A16 = A.astype(np.float16)
    # box-major rows: per chunk [x0,y0,x1,y1 | w0,h0,w1,h1 | c0,c1], each row
    # a contiguous n-vector
    xy = A16[..., PERM_XY].transpose(0, 1, 2, 4, 3)
    wh = A16[..., PERM_WH].transpose(0, 1, 2, 4, 3)
    cf = A16[..., PERM_CF].transpose(0, 1, 2, 4, 3)
    box = np.ascontiguousarray(np.concatenate([xy, wh, cf], axis=-2)).reshape(
        N_CORES, P, -1
    )
    cl = np.ascontiguousarray(A16[..., 10:30].transpose(0, 1, 2, 4, 3)).reshape(
        N_CORES, P, -1
    )
    return box, cl


def kernel(pred_tensor, target_tensor):
    global LAST_EXEC_NS, LAST_RESULT
    pred = np.asarray(pred_tensor).reshape(N_FULL * S * S, D)
    tgt = np.asarray(target_tensor).reshape(N_FULL * S * S, D)

    pb, pc = _prep(pred)
    tb, tc = _prep(tgt)

    in_maps = []
    for i in range(N_CORES):
        in_maps.append({"pbox": pb[i], "tbox": tb[i], "pcl": pc[i], "tcl": tc[i]})

    nc = _get_nc()
    trace = bool(os.environ.get("KERNEL_TRACE"))
    tmpdir = os.environ.get("KERNEL_TRACE_DIR") or None
    res = bass_utils.run_bass_kernel_spmd(
        nc, in_maps, core_ids=list(range(N_CORES)), trace=trace, tmpdir=tmpdir
    )
    LAST_RESULT = res
    if res.exec_time_ns is not None:
        LAST_EXEC_NS = res.exec_time_ns
    total = np.zeros(5, dtype=np.float64)
    for m in res.results:
        total += m["out"].astype(np.float64).sum(axis=0).reshape(N_CH, 5).sum(axis=0)
    losses = (total / float(N_FULL)).astype(np.float32)
    return losses


# revision 24
# speedup vs baseline: 1.2592x; 1.0131x over previous
"""YOLO-style loss kernel for Trainium2, SPMD over 8 NeuronCores.

Inputs (full): pred_tensor [32768,7,7,30] f32, target_tensor [32768,7,7,30] f32.
Output: np.ndarray shape (5,) f32 = (loss_xy, loss_wh, loss_obj, loss_noobj, loss_class).

Strategy: pure data parallel on batch dim; each core gets 4096 samples
(200704 cells). Host converts to fp16 and splits channels into fully
contiguous groups so the hot DVE ops coalesce into the 2x packed mode
(strided views of an interleaved [n,10] tile measure 1x or worse):
  - xy4  [n,4] cell-major (x0,y0,x1,y1)        both tensors
  - wh4  [n,4] cell-major (w0,h0,w1,h1)        both tensors
  - cf2  [n,2] cell-major (c0,c1)              both tensors
  - cls  [20,n] channel-major per chunk        both tensors
Per 392-cell chunk: IoU responsibility + five masked squared-diff partial
sums, fused on-chip. Weighted reductions run as premask-multiply (masks are
exactly 0/1) + in-place Square with accum_out on the scalar engine; the
class premask ANDs int32-reinterpreted fp16 pairs against a 0xFFFF mask;
reciprocal via the ~1cpe approx custom-DVE op. Each core returns a [128,20]
f32 partial-sum tile (5 losses x 4 chunks); host reduces and divides by N.
"""

import os
import sys

sys.path.insert(0, "/opt/trn_rl_repo")

import numpy as np

import concourse.bass as bass
import concourse.bacc as bacc
import concourse.tile as tile
from concourse import mybir
from concourse import bass_utils

F32 = mybir.dt.float32
F16 = mybir.dt.float16
I16 = mybir.dt.int16
I32 = mybir.dt.int32
ALU = mybir.AluOpType
ACT = mybir.ActivationFunctionType

S = 7
B = 2
C = 20
D = 30
N_FULL = 32768
N_CORES = 8
N_SHARD = N_FULL // N_CORES            # 4096 samples per core
R = N_SHARD * S * S                    # 200704 cells per core
P = 128                                # partitions
RP = R // P                            # 1568 cells per partition
NCK = 392                              # cells per partition per chunk
N_CH = RP // NCK                       # 4 chunks

PERM_XY = [0, 1, 5, 6]   # x0,y0,x1,y1
PERM_WH = [2, 3, 7, 8]   # w0,h0,w1,h1
PERM_CF = [4, 9]         # c0,c1


def _mk(ap, dims):
    """Rebuild the free dims of `ap` (keeping partition dim + offset) as
    `dims` = list of (step, count)."""
    new = [list(ap.ap[0])] + [[s, c] for s, c in dims]
    return bass.AP(tensor=ap.tensor, offset=ap.offset, ap=new)


def _ins(ap, pos, step, count):
    new = [list(x) for x in ap.ap]
    new.insert(pos, [step, count])
    return bass.AP(tensor=ap.tensor, offset=ap.offset, ap=new)


def build_program():
    nc = bacc.Bacc("TRN2", target_bir_lowering=False, debug=False)
    n = NCK

    def din(name, per_chunk):
        return nc.dram_tensor(name, [P, N_CH * per_chunk], F16, kind="ExternalInput")

    pbox, tbox = din("pbox", n * 10), din("tbox", n * 10)
    pcl, tcl = din("pcl", C * n), din("tcl", C * n)
    out = nc.dram_tensor("out", [P, 5 * N_CH], F32, kind="ExternalOutput")

    pbox_v = pbox.ap().rearrange("p (k a) -> p k a", k=N_CH, a=n * 10)
    tbox_v = tbox.ap().rearrange("p (k a) -> p k a", k=N_CH, a=n * 10)
    pcl_v = pcl.ap().rearrange("p (k c i) -> p k c i", k=N_CH, c=C, i=n)
    tcl_v = tcl.ap().rearrange("p (k c i) -> p k c i", k=N_CH, c=C, i=n)

    with tile.TileContext(nc) as tc:
        with (
            tc.tile_pool(name="raw", bufs=2) as raw,
            tc.tile_pool(name="tmp", bufs=1) as tmp,
            tc.tile_pool(name="persist", bufs=1) as persist,
        ):
            acc = persist.tile([P, 5 * N_CH], F32)

            for k in range(N_CH):
                # one block-major box DMA per tensor: [xy4(4n) | wh4(4n) | cf2(2n)]
                # contiguous inside the transfer, so every group view coalesces
                Bp = raw.tile([P, 10 * n], F16, tag="Bp")
                Bt = raw.tile([P, 10 * n], F16, tag="Bt")
                Pcl = raw.tile([P, C, n], F16, tag="Pcl")
                Tcl = raw.tile([P, C, n], F16, tag="Tcl")
                nc.sync.dma_start(out=Bp, in_=pbox_v[:, k])
                nc.sync.dma_start(out=Bt, in_=tbox_v[:, k])
                nc.sync.dma_start(out=Pcl, in_=pcl_v[:, k])
                nc.sync.dma_start(out=Tcl, in_=tcl_v[:, k])

                # box-major rows: x0,y0,x1,y1 | w0,h0,w1,h1 | c0,c1, each a
                # contiguous n-row, so every group op fully coalesces
                Pxy = Bp[:, 0:4 * n]
                Pwh = Bp[:, 4 * n:8 * n]
                Pcf = Bp[:, 8 * n:10 * n]
                Txy = Bt[:, 0:4 * n]
                Twh = Bt[:, 4 * n:8 * n]
                Tcf = Bt[:, 8 * n:10 * n]
                obj_src = Bt[:, 8 * n:9 * n]    # target c0 row, compact [P,n]

                def sqacc(dm, col):
                    # in-place square: ACT streams read-then-write per element,
                    # so out == in is safe and avoids junk tiles whose reuse
                    # would couple engines across chunks
                    nc.scalar.activation(
                        dm, dm, ACT.Square,
                        accum_out=acc[:, 5 * k + col:5 * k + col + 1],
                    )

                def class_block():
                    # class (channel-major [P,20,n]): AND the fp16 diffs
                    # against a 0xFFFF/0x0000 obj mask through int32 views —
                    # pair-packing halves the cost vs a 1x broadcast multiply
                    ffi = tmp.tile([P, n], I16, tag="ffi")
                    nc.scalar.activation(ffi, obj_src, ACT.Copy, scale=-1.0)
                    ff32 = ffi.bitcast(I32)                       # [P, n/2]
                    ff32b = _mk(ff32[:, 0], [(0, C), (1, n // 2)])
                    dcl = tmp.tile([P, C, n], F16, tag="dcl")
                    dmcl = tmp.tile([P, C, n], F16, tag="dmcl")
                    nc.vector.tensor_tensor(dcl, Tcl, Pcl, op=ALU.subtract)
                    nc.vector.tensor_tensor(
                        dmcl.bitcast(I32), dcl.bitcast(I32), ff32b,
                        op=ALU.bitwise_and,
                    )
                    sqacc(dmcl, 4)

                # on the final chunk run the class loss first: its inputs are
                # prefetched by then, and the big ACT square-accum no longer
                # sits alone in the kernel tail
                cls_done = k == N_CH - 1
                if cls_done:
                    class_block()

                # ---- IoU stage (coords scaled x7: corners 3.5*wh -+ xy) ----
                t1 = tmp.tile([P, 4, n], F16, tag="t1")
                nc.vector.tensor_scalar(t1, Pwh, 3.5, None, op0=ALU.mult)
                nl4 = tmp.tile([P, 4, n], F16, tag="nl4")    # -(7l) both boxes
                r4 = tmp.tile([P, 4, n], F16, tag="r4")      # 7r both boxes
                nc.vector.tensor_tensor(nl4, t1, Pxy, op=ALU.subtract)
                nc.vector.tensor_tensor(r4, t1, Pxy, op=ALU.add)

                # target corners, box0 only (x0,y0 / w0,h0 rows contiguous)
                txy0 = Bt[:, 0:2 * n]
                twh0 = Bt[:, 4 * n:6 * n]
                t2 = tmp.tile([P, 2, n], F16, tag="t2")
                nc.vector.tensor_scalar(t2, twh0, 3.5, None, op0=ALU.mult)
                nlt2 = tmp.tile([P, 2, n], F16, tag="nlt2")
                rt2 = tmp.tile([P, 2, n], F16, tag="rt2")
                nc.vector.tensor_tensor(nlt2, t2, txy0, op=ALU.subtract)
                nc.vector.tensor_tensor(rt2, t2, txy0, op=ALU.add)
                # rows (x,y,x,y): [boxdup step0][coord-row step n][cell step1]
                nlt2b = _mk(nlt2[:, 0, 0], [(0, 2), (n, 2), (1, n)])
                rt2b = _mk(rt2[:, 0, 0], [(0, 2), (n, 2), (1, n)])

                mln4 = tmp.tile([P, 4, n], F16, tag="mln4")
                mr4 = tmp.tile([P, 4, n], F16, tag="mr4")
                nc.vector.tensor_tensor(mln4, nl4, nlt2b, op=ALU.min)
                nc.vector.tensor_tensor(mr4, r4, rt2b, op=ALU.min)
                s4 = nl4  # dead, reuse
                nc.vector.tensor_tensor(s4, mln4, mr4, op=ALU.add)   # 7*(minr-maxl)
                cw4 = r4  # dead, reuse
                nc.vector.tensor_scalar(cw4, s4, 1.0 / 7.0, 0.0, op0=ALU.mult, op1=ALU.max)

                # per-box scalars, box-major [P,2,n]
                inter2 = tmp.tile([P, 2, n], F16, tag="inter2")
                areap2 = tmp.tile([P, 2, n], F16, tag="areap2")
                areat = tmp.tile([P, n], F16, tag="areat")
                cwx = cw4[:, 0:4:2, :]                           # x rows {0,2}
                cwy = cw4[:, 1:4:2, :]                           # y rows {1,3}
                nc.vector.tensor_tensor(inter2, cwx, cwy, op=ALU.mult)
                pw2 = _mk(Bp[:, 4 * n], [(2 * n, 2), (1, n)])    # w rows {0,2}
                ph2 = _mk(Bp[:, 5 * n], [(2 * n, 2), (1, n)])    # h rows {1,3}
                nc.vector.tensor_tensor(areap2, pw2, ph2, op=ALU.mult)
                nc.vector.tensor_tensor(areat, Bt[:, 4 * n:5 * n], Bt[:, 5 * n:6 * n], op=ALU.mult)

                u2h = tmp.tile([P, 2, n], F16, tag="u2h")
                u2 = tmp.tile([P, 2, n], F16, tag="u2")
                nc.vector.tensor_tensor(u2h, areap2, inter2, op=ALU.subtract)
                areatb = _ins(areat[:, :], 1, 0, 2)          # [box step0][cell step1]
                nc.vector.tensor_tensor(u2, u2h, areatb, op=ALU.add)

                # call the approx-reciprocal custom op directly with fp16
                # operands: the DVE converts fp16->fp32 at read BEFORE the
                # BITWISE_NOT seed, so the fp32-bit-layout trick still holds;
                # this keeps u2 a 2x fp16 add and drops the ACT downcast hop
                from concourse.dve_ops import (
                    RECIP_APPROX_FAST_CONSTS as _RC,
                    RECIPROCAL_APPROX_FAST as _RF,
                )
                rcp16 = tmp.tile([P, 2, n], F16, tag="rcp16")
                nc.vector._custom_dve(
                    _RF, out=rcp16, in0=u2,
                    s0=_RC["s0"], s1=_RC["s1"], imm2=_RC["imm2"],
                )
                iou2 = tmp.tile([P, 2, n], F16, tag="iou2")
                nc.vector.tensor_tensor(iou2, inter2, rcp16, op=ALU.mult)

                is1 = tmp.tile([P, n], F16, tag="is1")
                riou = tmp.tile([P, n], F16, tag="riou")
                nc.vector.tensor_tensor(is1, iou2[:, 1, :], iou2[:, 0, :], op=ALU.is_gt)
                nc.vector.tensor_tensor(riou, iou2[:, 1, :], iou2[:, 0, :], op=ALU.max)

                resp = tmp.tile([P, 2, n], F16, tag="resp")
                nc.vector.tensor_tensor(resp[:, 1, :], obj_src, is1, op=ALU.mult)
                nc.vector.tensor_tensor(resp[:, 0, :], obj_src, resp[:, 1, :], op=ALU.subtract)

                # ---- losses: premask (DVE) + in-place Square-with-accum (ACT)
                # xy and wh diffs share one [P,8,n] tile; a single broadcast
                # premask multiply covers both (broadcast dims do not break
                # the 2x packed mode), then two sqaccs split the accum columns
                d8 = tmp.tile([P, 8, n], F16, tag="d8")
                dm8 = tmp.tile([P, 8, n], F16, tag="dm8")
                nc.vector.tensor_tensor(d8[:, 0:4, :], Txy, Pxy, op=ALU.subtract)
                sp4 = tmp.tile([P, 4, n], F16, tag="sp4")
                st4 = tmp.tile([P, 4, n], F16, tag="st4")
                nc.scalar.activation(sp4, Pwh, ACT.Sqrt)
                nc.scalar.activation(st4, Twh, ACT.Sqrt)
                nc.vector.tensor_tensor(d8[:, 4:8, :], st4, sp4, op=ALU.subtract)
                # rows (r0,r0,r1,r1): [box n][coorddup 0][cell 1] — 3 free dims
                # (4 exceeds the ISA AP limit); broadcast does not break 2x
                resp4b = _mk(resp[:, 0, 0], [(n, 2), (0, 2), (1, n)])
                nc.vector.tensor_tensor(dm8[:, 0:4, :], d8[:, 0:4, :], resp4b, op=ALU.mult)
                nc.vector.tensor_tensor(dm8[:, 4:8, :], d8[:, 4:8, :], resp4b, op=ALU.mult)
                sqacc(dm8[:, 0:4, :], 0)
                sqacc(dm8[:, 4:8, :], 1)

                # obj conf vs responsible-iou, box-major [P,2,n]: conf rows
                # are compact so diff and premask both pack
                dc2 = tmp.tile([P, 2, n], F16, tag="dc2")
                dmc2 = tmp.tile([P, 2, n], F16, tag="dmc2")
                rioub = _ins(riou[:, :], 1, 0, 2)                 # [boxdup][cell]
                nc.vector.tensor_tensor(dc2, rioub, Pcf, op=ALU.subtract)
                nc.vector.tensor_tensor(dmc2, dc2, resp, op=ALU.mult)
                sqacc(dmc2, 2)

                # noobj conf: noobj*(tc-pc)^2 == ((tc*pc)-pc)^2 since tc in {0,1}
                m2 = tmp.tile([P, 2, n], F16, tag="m2")
                dmn2 = tmp.tile([P, 2, n], F16, tag="dmn2")
                nc.vector.tensor_tensor(m2, Tcf, Pcf, op=ALU.mult)
                nc.vector.tensor_tensor(dmn2, m2, Pcf, op=ALU.subtract)
                sqacc(dmn2, 3)

                if not cls_done:
                    class_block()

            nc.sync.dma_start(out=out.ap(), in_=acc)

    nc.compile()
    return nc


_nc_cache = None
LAST_EXEC_NS = None
LAST_RESULT = None


def _get_nc():
    global _nc_cache
    if _nc_cache is None:
        _nc_cache = build_program()
    return _nc_cache


def _prep(full):
    """[N*S*S, 30] f32 -> per-core fp16 (box blocks [k][xy4|wh4|cf2], cls)."""
    A = np.asarray(full, dtype=np.float32).reshape(N_CORES, P, N_CH, NCK, D)
    A16 = A.astype(np.float16)
    # box-major rows: per chunk [x0,y0,x1,y1 | w0,h0,w1,h1 | c0,c1], each row
    # a contiguous n-vector
    xy = A16[..., PERM_XY].transpose(0, 1, 2, 4, 3)
    wh = A16[..., PERM_WH].transpose(0, 1, 2, 4, 3)
    cf = A16[..., PERM_CF].transpose(0, 1, 2, 4, 3)
    box = np.ascontiguousarray(np.concatenate([xy, wh, cf], axis=-2)).reshape(
        N_CORES, P, -1
    )
    cl = np.ascontiguousarray(A16[..., 10:30].transpose(0, 1, 2, 4, 3)).reshape(
        N_CORES, P, -1
    )
    return box, cl


def kernel(pred_tensor, target_tensor):
    global LAST_EXEC_NS, LAST_RESULT
    pred = np.asarray(pred_tensor).reshape(N_FULL * S * S, D)
    tgt = np.asarray(target_tensor).reshape(N_FULL * S * S, D)

    pb, pc = _prep(pred)
    tb, tc = _prep(tgt)

    in_maps = []
    for i in range(N_CORES):
        in_maps.append({"pbox": pb[i], "tbox": tb[i], "pcl": pc[i], "tcl": tc[i]})

    nc = _get_nc()
    trace = bool(os.environ.get("KERNEL_TRACE"))
    tmpdir = os.environ.get("KERNEL_TRACE_DIR") or None
    res = bass_utils.run_bass_kernel_spmd(
        nc, in_maps, core_ids=list(range(N_CORES)), trace=trace, tmpdir=tmpdir
    )
    LAST_RESULT = res
    if res.exec_time_ns is not None:
        LAST_EXEC_NS = res.exec_time_ns
    total = np.zeros(5, dtype=np.float64)
    for m in res.results:
        total += m["out"].astype(np.float64).sum(axis=0).reshape(N_CH, 5).sum(axis=0)
    losses = (total / float(N_FULL)).astype(np.float32)
    return losses


# revision 25
# speedup vs baseline: 1.3132x; 1.0429x over previous
"""YOLO-style loss kernel for Trainium2, SPMD over 8 NeuronCores.

Inputs (full): pred_tensor [32768,7,7,30] f32, target_tensor [32768,7,7,30] f32.
Output: np.ndarray shape (5,) f32 = (loss_xy, loss_wh, loss_obj, loss_noobj, loss_class).

Strategy: pure data parallel on batch dim; each core gets 4096 samples
(200704 cells). Host converts to fp16 and splits channels into fully
contiguous groups so the hot DVE ops coalesce into the 2x packed mode
(strided views of an interleaved [n,10] tile measure 1x or worse):
  - xy4  [n,4] cell-major (x0,y0,x1,y1)        both tensors
  - wh4  [n,4] cell-major (w0,h0,w1,h1)        both tensors
  - cf2  [n,2] cell-major (c0,c1)              both tensors
  - cls  [20,n] channel-major per chunk        both tensors
Per 392-cell chunk: IoU responsibility + five masked squared-diff partial
sums, fused on-chip. Weighted reductions run as premask-multiply (masks are
exactly 0/1) + in-place Square with accum_out on the scalar engine; the
class premask ANDs int32-reinterpreted fp16 pairs against a 0xFFFF mask;
reciprocal via the ~1cpe approx custom-DVE op. Each core returns a [128,20]
f32 partial-sum tile (5 losses x 4 chunks); host reduces and divides by N.
"""

import os
import sys

sys.path.insert(0, "/opt/trn_rl_repo")

import numpy as np

import concourse.bass as bass
import concourse.bacc as bacc
import concourse.tile as tile
from concourse import mybir
from concourse import bass_utils

F32 = mybir.dt.float32
F16 = mybir.dt.float16
I16 = mybir.dt.int16
I32 = mybir.dt.int32
ALU = mybir.AluOpType
ACT = mybir.ActivationFunctionType

S = 7
B = 2
C = 20
D = 30
N_FULL = 32768
N_CORES = 8
N_SHARD = N_FULL // N_CORES            # 4096 samples per core
R = N_SHARD * S * S                    # 200704 cells per core
P = 128                                # partitions
RP = R // P                            # 1568 cells per partition
NCK = 392                              # cells per partition per chunk
N_CH = RP // NCK                       # 4 chunks

PERM_XY = [0, 1, 5, 6]   # x0,y0,x1,y1
PERM_WH = [2, 3, 7, 8]   # w0,h0,w1,h1
PERM_CF = [4, 9]         # c0,c1


def _mk(ap, dims):
    """Rebuild the free dims of `ap` (keeping partition dim + offset) as
    `dims` = list of (step, count)."""
    new = [list(ap.ap[0])] + [[s, c] for s, c in dims]
    return bass.AP(tensor=ap.tensor, offset=ap.offset, ap=new)


def _ins(ap, pos, step, count):
    new = [list(x) for x in ap.ap]
    new.insert(pos, [step, count])
    return bass.AP(tensor=ap.tensor, offset=ap.offset, ap=new)


def build_program():
    nc = bacc.Bacc("TRN2", target_bir_lowering=False, debug=False)
    n = NCK

    def din(name, per_chunk):
        return nc.dram_tensor(name, [P, N_CH * per_chunk], F16, kind="ExternalInput")

    pbox, tbox = din("pbox", n * 10), din("tbox", n * 10)
    pcl, tcl = din("pcl", C * n), din("tcl", C * n)
    out = nc.dram_tensor("out", [P, 5 * N_CH], F32, kind="ExternalOutput")

    pbox_v = pbox.ap().rearrange("p (k a) -> p k a", k=N_CH, a=n * 10)
    tbox_v = tbox.ap().rearrange("p (k a) -> p k a", k=N_CH, a=n * 10)
    pcl_v = pcl.ap().rearrange("p (k c i) -> p k c i", k=N_CH, c=C, i=n)
    tcl_v = tcl.ap().rearrange("p (k c i) -> p k c i", k=N_CH, c=C, i=n)

    with tile.TileContext(nc) as tc:
        with (
            tc.tile_pool(name="raw", bufs=2) as raw,
            tc.tile_pool(name="tmp", bufs=1) as tmp,
            tc.tile_pool(name="persist", bufs=1) as persist,
        ):
            acc = persist.tile([P, 5 * N_CH], F32)

            for k in range(N_CH):
                # one block-major box DMA per tensor: [xy4(4n) | wh4(4n) | cf2(2n)]
                # contiguous inside the transfer, so every group view coalesces
                Bp = raw.tile([P, 10 * n], F16, tag="Bp")
                Bt = raw.tile([P, 10 * n], F16, tag="Bt")
                Pcl = raw.tile([P, C, n], F16, tag="Pcl")
                Tcl = raw.tile([P, C, n], F16, tag="Tcl")
                nc.sync.dma_start(out=Bp, in_=pbox_v[:, k])
                nc.sync.dma_start(out=Bt, in_=tbox_v[:, k])
                nc.sync.dma_start(out=Pcl, in_=pcl_v[:, k])
                nc.sync.dma_start(out=Tcl, in_=tcl_v[:, k])

                # box-major rows: x0,y0,x1,y1 | w0,h0,w1,h1 | c0,c1, each a
                # contiguous n-row, so every group op fully coalesces
                Pxy = Bp[:, 0:4 * n]
                Pwh = Bp[:, 4 * n:8 * n]
                Pcf = Bp[:, 8 * n:10 * n]
                Txy = Bt[:, 0:4 * n]
                Twh = Bt[:, 4 * n:8 * n]
                Tcf = Bt[:, 8 * n:10 * n]
                obj_src = Bt[:, 8 * n:9 * n]    # target c0 row, compact [P,n]

                def sqacc(dm, col):
                    # in-place square: ACT streams read-then-write per element,
                    # so out == in is safe and avoids junk tiles whose reuse
                    # would couple engines across chunks
                    nc.scalar.activation(
                        dm, dm, ACT.Square,
                        accum_out=acc[:, 5 * k + col:5 * k + col + 1],
                    )

                def class_block():
                    # class (channel-major [P,20,n]): AND the fp16 diffs
                    # against a 0xFFFF/0x0000 obj mask through int32 views —
                    # pair-packing halves the cost vs a 1x broadcast multiply
                    ffi = tmp.tile([P, n], I16, tag="ffi")
                    nc.scalar.activation(ffi, obj_src, ACT.Copy, scale=-1.0)
                    ff32 = ffi.bitcast(I32)                       # [P, n/2]
                    ff32b = _mk(ff32[:, 0], [(0, C), (1, n // 2)])
                    dcl = tmp.tile([P, C, n], F16, tag="dcl")
                    dmcl = tmp.tile([P, C, n], F16, tag="dmcl")
                    nc.vector.tensor_tensor(dcl, Tcl, Pcl, op=ALU.subtract)
                    nc.vector.tensor_tensor(
                        dmcl.bitcast(I32), dcl.bitcast(I32), ff32b,
                        op=ALU.bitwise_and,
                    )
                    sqacc(dmcl, 4)

                # ---- IoU stage (coords scaled x7: corners 3.5*wh -+ xy) ----
                # single-input scale/clamp ops ride the scalar engine (slack)
                t1 = tmp.tile([P, 4, n], F16, tag="t1")
                nc.scalar.activation(t1, Pwh, ACT.Copy, scale=3.5)
                nl4 = tmp.tile([P, 4, n], F16, tag="nl4")    # -(7l) both boxes
                r4 = tmp.tile([P, 4, n], F16, tag="r4")      # 7r both boxes
                nc.vector.tensor_tensor(nl4, t1, Pxy, op=ALU.subtract)
                nc.vector.tensor_tensor(r4, t1, Pxy, op=ALU.add)

                # target corners, box0 only (x0,y0 / w0,h0 rows contiguous)
                txy0 = Bt[:, 0:2 * n]
                twh0 = Bt[:, 4 * n:6 * n]
                t2 = tmp.tile([P, 2, n], F16, tag="t2")
                nc.scalar.activation(t2, twh0, ACT.Copy, scale=3.5)
                nlt2 = tmp.tile([P, 2, n], F16, tag="nlt2")
                rt2 = tmp.tile([P, 2, n], F16, tag="rt2")
                nc.vector.tensor_tensor(nlt2, t2, txy0, op=ALU.subtract)
                nc.vector.tensor_tensor(rt2, t2, txy0, op=ALU.add)
                # rows (x,y,x,y): [boxdup step0][coord-row step n][cell step1]
                nlt2b = _mk(nlt2[:, 0, 0], [(0, 2), (n, 2), (1, n)])
                rt2b = _mk(rt2[:, 0, 0], [(0, 2), (n, 2), (1, n)])

                mln4 = tmp.tile([P, 4, n], F16, tag="mln4")
                mr4 = tmp.tile([P, 4, n], F16, tag="mr4")
                nc.vector.tensor_tensor(mln4, nl4, nlt2b, op=ALU.min)
                nc.vector.tensor_tensor(mr4, r4, rt2b, op=ALU.min)
                s4 = nl4  # dead, reuse
                nc.vector.tensor_tensor(s4, mln4, mr4, op=ALU.add)   # 7*(minr-maxl)
                cw4 = r4  # dead, reuse
                nc.scalar.activation(cw4, s4, ACT.Relu, scale=1.0 / 7.0)

                # class block here: ~8.5us of independent DVE work overlapping
                # the ACT cw4 (mid-chunk, so its DMAs are long since landed —
                # unlike class-first-at-chunk-top, which starved the head)
                class_block()

                # per-box scalars, box-major [P,2,n]
                inter2 = tmp.tile([P, 2, n], F16, tag="inter2")
                areap2 = tmp.tile([P, 2, n], F16, tag="areap2")
                areat = tmp.tile([P, n], F16, tag="areat")
                cwx = cw4[:, 0:4:2, :]                           # x rows {0,2}
                cwy = cw4[:, 1:4:2, :]                           # y rows {1,3}
                nc.vector.tensor_tensor(inter2, cwx, cwy, op=ALU.mult)
                pw2 = _mk(Bp[:, 4 * n], [(2 * n, 2), (1, n)])    # w rows {0,2}
                ph2 = _mk(Bp[:, 5 * n], [(2 * n, 2), (1, n)])    # h rows {1,3}
                nc.vector.tensor_tensor(areap2, pw2, ph2, op=ALU.mult)
                nc.vector.tensor_tensor(areat, Bt[:, 4 * n:5 * n], Bt[:, 5 * n:6 * n], op=ALU.mult)

                u2h = tmp.tile([P, 2, n], F16, tag="u2h")
                u2 = tmp.tile([P, 2, n], F16, tag="u2")
                nc.vector.tensor_tensor(u2h, areap2, inter2, op=ALU.subtract)
                areatb = _ins(areat[:, :], 1, 0, 2)          # [box step0][cell step1]
                nc.vector.tensor_tensor(u2, u2h, areatb, op=ALU.add)

                # call the approx-reciprocal custom op directly with fp16
                # operands: the DVE converts fp16->fp32 at read BEFORE the
                # BITWISE_NOT seed, so the fp32-bit-layout trick still holds;
                # this keeps u2 a 2x fp16 add and drops the ACT downcast hop
                from concourse.dve_ops import (
                    RECIP_APPROX_FAST_CONSTS as _RC,
                    RECIPROCAL_APPROX_FAST as _RF,
                )
                rcp16 = tmp.tile([P, 2, n], F16, tag="rcp16")
                nc.vector._custom_dve(
                    _RF, out=rcp16, in0=u2,
                    s0=_RC["s0"], s1=_RC["s1"], imm2=_RC["imm2"],
                )
                iou2 = tmp.tile([P, 2, n], F16, tag="iou2")
                nc.vector.tensor_tensor(iou2, inter2, rcp16, op=ALU.mult)

                is1 = tmp.tile([P, n], F16, tag="is1")
                riou = tmp.tile([P, n], F16, tag="riou")
                nc.vector.tensor_tensor(is1, iou2[:, 1, :], iou2[:, 0, :], op=ALU.is_gt)
                nc.vector.tensor_tensor(riou, iou2[:, 1, :], iou2[:, 0, :], op=ALU.max)

                resp = tmp.tile([P, 2, n], F16, tag="resp")
                nc.vector.tensor_tensor(resp[:, 1, :], obj_src, is1, op=ALU.mult)
                nc.vector.tensor_tensor(resp[:, 0, :], obj_src, resp[:, 1, :], op=ALU.subtract)

                # ---- losses: premask (DVE) + in-place Square-with-accum (ACT)
                # xy and wh diffs share one [P,8,n] tile; a single broadcast
                # premask multiply covers both (broadcast dims do not break
                # the 2x packed mode), then two sqaccs split the accum columns
                d8 = tmp.tile([P, 8, n], F16, tag="d8")
                dm8 = tmp.tile([P, 8, n], F16, tag="dm8")
                nc.vector.tensor_tensor(d8[:, 0:4, :], Txy, Pxy, op=ALU.subtract)
                sp4 = tmp.tile([P, 4, n], F16, tag="sp4")
                st4 = tmp.tile([P, 4, n], F16, tag="st4")
                nc.scalar.activation(sp4, Pwh, ACT.Sqrt)
                nc.scalar.activation(st4, Twh, ACT.Sqrt)
                nc.vector.tensor_tensor(d8[:, 4:8, :], st4, sp4, op=ALU.subtract)
                # rows (r0,r0,r1,r1): [box n][coorddup 0][cell 1] — 3 free dims
                # (4 exceeds the ISA AP limit); broadcast does not break 2x
                resp4b = _mk(resp[:, 0, 0], [(n, 2), (0, 2), (1, n)])
                nc.vector.tensor_tensor(dm8[:, 0:4, :], d8[:, 0:4, :], resp4b, op=ALU.mult)
                nc.vector.tensor_tensor(dm8[:, 4:8, :], d8[:, 4:8, :], resp4b, op=ALU.mult)
                sqacc(dm8[:, 0:4, :], 0)
                sqacc(dm8[:, 4:8, :], 1)

                # obj conf vs responsible-iou, box-major [P,2,n]: conf rows
                # are compact so diff and premask both pack
                dc2 = tmp.tile([P, 2, n], F16, tag="dc2")
                dmc2 = tmp.tile([P, 2, n], F16, tag="dmc2")
                rioub = _ins(riou[:, :], 1, 0, 2)                 # [boxdup][cell]
                nc.vector.tensor_tensor(dc2, rioub, Pcf, op=ALU.subtract)
                nc.vector.tensor_tensor(dmc2, dc2, resp, op=ALU.mult)
                sqacc(dmc2, 2)

                # noobj conf: noobj*(tc-pc)^2 == ((tc*pc)-pc)^2 since tc in {0,1}
                m2 = tmp.tile([P, 2, n], F16, tag="m2")
                dmn2 = tmp.tile([P, 2, n], F16, tag="dmn2")
                nc.vector.tensor_tensor(m2, Tcf, Pcf, op=ALU.mult)
                nc.vector.tensor_tensor(dmn2, m2, Pcf, op=ALU.subtract)
                sqacc(dmn2, 3)


            nc.sync.dma_start(out=out.ap(), in_=acc)

    nc.compile()
    return nc


_nc_cache = None
LAST_EXEC_NS = None
LAST_RESULT = None


def _get_nc():
    global _nc_cache
    if _nc_cache is None:
        _nc_cache = build_program()
    return _nc_cache


def _prep(full):
    """[N*S*S, 30] f32 -> per-core fp16 (box blocks [k][xy4|wh4|cf2], cls)."""
    A = np.asarray(full, dtype=np.float32).reshape(N_CORES, P, N_CH, NCK, D)
    A16 = A.astype(np.float16)
    # box-major rows: per chunk [x0,y0,x1,y1 | w0,h0,w1,h1 | c0,c1], each row
    # a contiguous n-vector
    xy = A16[..., PERM_XY].transpose(0, 1, 2, 4, 3)
    wh = A16[..., PERM_WH].transpose(0, 1, 2, 4, 3)
    cf = A16[..., PERM_CF].transpose(0, 1, 2, 4, 3)
    box = np.ascontiguousarray(np.concatenate([xy, wh, cf], axis=-2)).reshape(
        N_CORES, P, -1
    )
    cl = np.ascontiguousarray(A16[..., 10:30].transpose(0, 1, 2, 4, 3)).reshape(
        N_CORES, P, -1
    )
    return box, cl


def kernel(pred_tensor, target_tensor):
    global LAST_EXEC_NS, LAST_RESULT
    pred = np.asarray(pred_tensor).reshape(N_FULL * S * S, D)
    tgt = np.asarray(target_tensor).reshape(N_FULL * S * S, D)

    pb, pc = _prep(pred)
    tb, tc = _prep(tgt)

    in_maps = []
    for i in range(N_CORES):
        in_maps.append({"pbox": pb[i], "tbox": tb[i], "pcl": pc[i], "tcl": tc[i]})

    nc = _get_nc()
    trace = bool(os.environ.get("KERNEL_TRACE"))
    tmpdir = os.environ.get("KERNEL_TRACE_DIR") or None
    res = bass_utils.run_bass_kernel_spmd(
        nc, in_maps, core_ids=list(range(N_CORES)), trace=trace, tmpdir=tmpdir
    )
    LAST_RESULT = res
    if res.exec_time_ns is not None:
        LAST_EXEC_NS = res.exec_time_ns
    total = np.zeros(5, dtype=np.float64)
    for m in res.results:
        total += m["out"].astype(np.float64).sum(axis=0).reshape(N_CH, 5).sum(axis=0)
    losses = (total / float(N_FULL)).astype(np.float32)
    return losses
